# revision 1
# baseline (speedup 1.0000x reference)
"""Routed-LoRA linear layer (moe_routing) on 8 trn2 NeuronCores.

Math (per token t):
  out[t, :] = W @ x[t] + b + 2.0 * sum_n mask[n, t] * (B_n @ (A_n @ x[t]))

Strategy:
  - Data-parallel over B*T = 65536 tokens: 8192 tokens per core.
  - All operand transposes are done host-side (numpy marshaling) so the
    device only ever streams contiguous, partition-friendly layouts:
      xt  [D_IN, TOK]  = x-shard transposed      (contraction dim major)
      wt  [D_IN, D_OUT] = W.T
      at  [D_IN, NR]    = fused-A.T
      btr [NR, D_OUT]   = fused-B.T
      msk [NR, TOK]     = routing mask expanded to rank dim, pre-scaled
  - fp32r matmuls (full PE rate at N=512), LoRA delta accumulated into the
    same PSUM bank as the base matmul, bias added during PSUM->SBUF copy.
"""

import numpy as np

import concourse.bass as bass
from concourse import bacc
import concourse.mybir as mybir
import concourse.tile as tile
from concourse.bass_utils import run_bass_kernel_spmd

N_CORES = 8
B, T = 8, 8192
D_IN = 1024
D_OUT = 1024
N_ADAPT, R = 4, 16
NR = N_ADAPT * R  # 64
SCALING = 32.0 / 16.0

TOK = B * T // N_CORES  # 8192 tokens per core
SUP = 512               # tokens per supertile
N_SUP = TOK // SUP      # 16
SUB = 128               # tokens per matmul M-tile
N_SUB = SUP // SUB      # 4
P = 128
KC = D_IN // P          # 8 contraction chunks
NB = D_OUT // 512       # 2 PSUM-bank column halves

F32 = mybir.dt.float32
F32R = mybir.dt.float32r


def build_bass(nrep=1, xp_bufs=3, pso_bufs=2, n_inner=False, split_bias=False):
    nc = bacc.Bacc(
        "TRN2", target_bir_lowering=False, debug=False, num_devices=N_CORES
    )

    xt_d = nc.dram_tensor("xt", [D_IN, TOK], F32R, kind="ExternalInput")
    wt_d = nc.dram_tensor("wt", [D_IN, D_OUT], F32R, kind="ExternalInput")
    at_d = nc.dram_tensor("at", [D_IN, NR], F32R, kind="ExternalInput")
    bt_d = nc.dram_tensor("btr", [NR, D_OUT], F32R, kind="ExternalInput")
    bias_d = nc.dram_tensor("bias", [D_OUT], F32, kind="ExternalInput")
    msk_d = nc.dram_tensor("msk", [NR, TOK], F32, kind="ExternalInput")
    out_d = nc.dram_tensor("out", [TOK, D_OUT], F32, kind="ExternalOutput")

    xt_r = xt_d.ap().rearrange("(kc p) t -> p kc t", p=P)
    wt_r = wt_d.ap().rearrange("(kc p) n -> p kc n", p=P)
    at_r = at_d.ap().rearrange("(kc p) j -> p kc j", p=P)
    out_r = out_d.ap().rearrange("(s q p) n -> s p q n", q=N_SUB, p=P)
    bias_bcast = bass.AP(
        tensor=bias_d, offset=0, ap=[[0, P], [1, D_OUT]]
    )

    with tile.TileContext(nc) as tc:
        with (
            tc.tile_pool(name="const", bufs=1) as const,
            tc.tile_pool(name="xp", bufs=xp_bufs) as xp,
            tc.tile_pool(name="sp", bufs=2) as sp,
            tc.tile_pool(name="op", bufs=2) as op,
            tc.tile_pool(name="pss", bufs=2, space="PSUM") as pss,
            tc.tile_pool(name="pso", bufs=pso_bufs, space="PSUM") as pso,
        ):
            w_sb = const.tile([P, KC, D_OUT], F32R)
            a_sb = const.tile([P, KC, NR], F32R)
            bt_sb = const.tile([NR, D_OUT], F32R)
            b_sb = const.tile([P, D_OUT], F32)
            m_sb = const.tile([NR, TOK], F32)
            # Preload order matters for startup latency: the first s-pass
            # matmuls need a_sb + x0 (sync queue), the first main matmuls
            # need w chunk 0 (scalar queue, split per-k so MMs start after
            # ~1.4us instead of waiting for the whole 4MB W load).
            nc.sync.dma_start(out=a_sb[:], in_=at_r)
            nc.scalar.dma_start(out=bt_sb[:], in_=bt_d.ap())
            for k in range(KC):
                nc.scalar.dma_start(out=w_sb[:, k, :], in_=wt_r[:, k, :])
            nc.gpsimd.dma_start(out=b_sb[:], in_=bias_bcast)

            for _rep in range(nrep):
                for s in range(N_SUP):
                    t0 = s * SUP
                    x_sb = xp.tile([P, KC, SUP], F32R, tag="x")
                    nc.sync.dma_start(
                        out=x_sb[:], in_=xt_r[:, :, t0 : t0 + SUP]
                    )
                    if _rep == 0:
                        # stream the mask in per-supertile so the first
                        # mask-multiply isn't gated on a monolithic 2MB load
                        nc.sync.dma_start(
                            out=m_sb[:, t0 : t0 + SUP],
                            in_=msk_d.ap()[:, t0 : t0 + SUP],
                        )

                    # s.T = fused_A @ x.T for this supertile: [NR, SUP]
                    s_ps = pss.tile([NR, SUP], F32, tag="sps")
                    for k in range(KC):
                        nc.tensor.matmul(
                            s_ps[:],
                            a_sb[:, k, :],
                            x_sb[:, k, :],
                            start=(k == 0),
                            stop=(k == KC - 1),
                        )
                    sm_sb = sp.tile([NR, SUP], F32R, tag="sm")
                    nc.vector.tensor_mul(
                        sm_sb[:], s_ps[:], m_sb[:, t0 : t0 + SUP]
                    )

                    o_sb = op.tile([P, N_SUB, D_OUT], F32, tag="o")
                    for q in range(N_SUB):
                        ts = q * SUB
                        o_ps = pso.tile([P, D_OUT], F32, tag="ops")
                        if n_inner:
                            for k in range(KC):
                                for n in range(NB):
                                    nsl = slice(n * 512, (n + 1) * 512)
                                    nc.tensor.matmul(
                                        o_ps[:, nsl],
                                        x_sb[:, k, ts : ts + SUB],
                                        w_sb[:, k, nsl],
                                        start=(k == 0),
                                        stop=False,
                                        skip_group_check=True,
                                    )
                            for n in range(NB):
                                nsl = slice(n * 512, (n + 1) * 512)
                                nc.tensor.matmul(
                                    o_ps[:, nsl],
                                    sm_sb[:, ts : ts + SUB],
                                    bt_sb[:, nsl],
                                    start=False,
                                    stop=True,
                                    skip_group_check=True,
                                )
                        else:
                            for n in range(NB):
                                nsl = slice(n * 512, (n + 1) * 512)
                                for k in range(KC):
                                    nc.tensor.matmul(
                                        o_ps[:, nsl],
                                        x_sb[:, k, ts : ts + SUB],
                                        w_sb[:, k, nsl],
                                        start=(k == 0),
                                        stop=False,
                                    )
                                nc.tensor.matmul(
                                    o_ps[:, nsl],
                                    sm_sb[:, ts : ts + SUB],
                                    bt_sb[:, nsl],
                                    start=False,
                                    stop=True,
                                )
                        if split_bias:
                            for n in range(NB):
                                nsl = slice(n * 512, (n + 1) * 512)
                                nc.vector.tensor_add(
                                    o_sb[:, q, nsl], o_ps[:, nsl], b_sb[:, nsl]
                                )
                        else:
                            nc.vector.tensor_add(o_sb[:, q, :], o_ps[:], b_sb[:])
                    nc.scalar.dma_start(out=out_r[s], in_=o_sb[:])

    nc.compile()
    return nc


_NC_CACHE = None


def _get_nc():
    global _NC_CACHE
    if _NC_CACHE is None:
        _NC_CACHE = build_bass()
    return _NC_CACHE


def make_in_maps(x, W, b, lora_A, lora_B, masks):
    x = np.ascontiguousarray(x, dtype=np.float32)
    W = np.ascontiguousarray(W, dtype=np.float32)
    b = np.ascontiguousarray(b, dtype=np.float32)
    lora_A = np.ascontiguousarray(lora_A, dtype=np.float32)
    lora_B = np.ascontiguousarray(lora_B, dtype=np.float32)
    masks = np.ascontiguousarray(masks, dtype=np.float32)

    x_flat = x.reshape(B * T, D_IN)
    A_flat = lora_A.reshape(NR, D_IN)
    B_flat = lora_B.transpose(1, 0, 2).reshape(D_OUT, NR)

    wt = np.ascontiguousarray(W.T)            # [D_IN, D_OUT]
    at = np.ascontiguousarray(A_flat.T)       # [D_IN, NR]
    btr = np.ascontiguousarray(B_flat.T)      # [NR, D_OUT]

    m_full = masks[..., 0].reshape(N_ADAPT, B * T) * np.float32(SCALING)
    m_exp = np.repeat(m_full, R, axis=0)      # [NR, B*T]

    in_maps = []
    for c in range(N_CORES):
        sl = slice(c * TOK, (c + 1) * TOK)
        in_maps.append(
            {
                "xt": np.ascontiguousarray(x_flat[sl].T),
                "wt": wt,
                "at": at,
                "btr": btr,
                "bias": b,
                "msk": np.ascontiguousarray(m_exp[:, sl]),
            }
        )
    return in_maps


def kernel(x, W, b, lora_A, lora_B, masks):
    nc = _get_nc()
    in_maps = make_in_maps(x, W, b, lora_A, lora_B, masks)
    res = run_bass_kernel_spmd(nc, in_maps, core_ids=list(range(N_CORES)))
    out = np.concatenate([r["out"] for r in res.results], axis=0)
    out = out.reshape(B, T, D_OUT)
    return out



# revision 2
# speedup vs baseline: 1.4608x; 1.4608x over previous
"""Routed-LoRA linear layer (moe_routing) on 8 trn2 NeuronCores.

Math (per token t):
  out[t, :] = W @ x[t] + b + 2.0 * sum_n mask[n, t] * (B_n @ (A_n @ x[t]))

Strategy:
  - Data-parallel over B*T = 65536 tokens: 8192 tokens per core.
  - All heavy matmuls run in fp8e4m3 with perf_mode=DoubleRow (2 k-tiles
    packed per instruction, K=256 contraction per matmul, half cycle cost
    per output row vs full-rate dtypes).
  - Precision is recovered with an error-compensated split, all terms
    sharing a single 2^7 scale (on the W/A/B side) so they accumulate
    into one PSUM group:
      main : (x_hi + x_lo) @ W_hi + x_hi @ W_lo      (x_lo@W_lo dropped)
      s    : x_hi @ (A_hi + A_lo)                     (rank-64 fused A)
      delta: sm @ (B_hi + B_lo)  -- B hi/lo ride the two DoubleRow j-slots
    where *_hi = fp8(v), *_lo = fp8(v - v_hi). sm = (s * mask) quantized
    to fp8 on the fly by the DVE mask-multiply.
  - Final PSUM->SBUF copy fuses descale (2^-7) and bias add in a single
    DVE scalar_tensor_tensor.
"""

import numpy as np
import ml_dtypes

import concourse.bass as bass
from concourse import bacc
import concourse.mybir as mybir
import concourse.tile as tile
from concourse.bass_utils import run_bass_kernel_spmd

N_CORES = 8
B, T = 8, 8192
D_IN = 1024
D_OUT = 1024
N_ADAPT, R = 4, 16
NR = N_ADAPT * R  # 64
SCALING = 32.0 / 16.0

TOK = B * T // N_CORES  # 8192 tokens per core
SUP = 512               # tokens per supertile
N_SUP = TOK // SUP      # 16
SUB = 128               # tokens per matmul M-tile
N_SUB = SUP // SUB      # 4
P = 128
KP = D_IN // 256        # 4 DoubleRow contraction chunks (256 each)
NB = D_OUT // 512       # 2 PSUM-bank column halves

SA = 2.0 ** 7           # weight-side scale so W/A/B fp8 values are ~N(0, 2.56)

F32 = mybir.dt.float32
F8 = mybir.dt.float8e4
NPF8 = ml_dtypes.float8_e4m3
DR = mybir.MatmulPerfMode.DoubleRow


def build_bass():
    nc = bacc.Bacc(
        "TRN2", target_bir_lowering=False, debug=False, num_devices=N_CORES
    )

    xh_d = nc.dram_tensor("xh", [KP, 2, P, TOK], F8, kind="ExternalInput")
    xl_d = nc.dram_tensor("xl", [KP, 2, P, TOK], F8, kind="ExternalInput")
    wh_d = nc.dram_tensor("wh", [KP, 2, P, D_OUT], F8, kind="ExternalInput")
    wl_d = nc.dram_tensor("wl", [KP, 2, P, D_OUT], F8, kind="ExternalInput")
    ah_d = nc.dram_tensor("ah", [KP, 2, P, NR], F8, kind="ExternalInput")
    al_d = nc.dram_tensor("al", [KP, 2, P, NR], F8, kind="ExternalInput")
    btp_d = nc.dram_tensor("btp", [NR, 2, D_OUT], F8, kind="ExternalInput")
    bias_d = nc.dram_tensor("bias", [D_OUT], F32, kind="ExternalInput")
    msk_d = nc.dram_tensor("msk", [NR, TOK], F8, kind="ExternalInput")
    out_d = nc.dram_tensor("out", [TOK, D_OUT], F32, kind="ExternalOutput")

    xh_r = xh_d.ap().rearrange("c j p t -> p c j t")
    xl_r = xl_d.ap().rearrange("c j p t -> p c j t")
    wh_r = wh_d.ap().rearrange("c j p n -> p c j n")
    wl_r = wl_d.ap().rearrange("c j p n -> p c j n")
    ah_r = ah_d.ap().rearrange("c j p r -> p c j r")
    al_r = al_d.ap().rearrange("c j p r -> p c j r")
    out_r = out_d.ap().rearrange("(s q p) n -> s p q n", q=N_SUB, p=P)
    bias_bcast = bass.AP(tensor=bias_d, offset=0, ap=[[0, P], [1, D_OUT]])

    with tile.TileContext(nc) as tc:
        with (
            tc.tile_pool(name="const", bufs=1) as const,
            tc.tile_pool(name="xp", bufs=3) as xp,
            tc.tile_pool(name="op", bufs=2) as op,
            tc.tile_pool(name="pss", bufs=2, space="PSUM") as pss,
            tc.tile_pool(name="pso", bufs=2, space="PSUM") as pso,
        ):
            wh_sb = const.tile([P, KP, 2, D_OUT], F8)
            wl_sb = const.tile([P, KP, 2, D_OUT], F8)
            ah_sb = const.tile([P, KP, 2, NR], F8)
            al_sb = const.tile([P, KP, 2, NR], F8)
            btp_sb = const.tile([NR, 2, D_OUT], F8)
            b_sb = const.tile([P, D_OUT], F32)
            m_sb = const.tile([NR, TOK], F8)
            sm_sb = const.tile([NR, 2, SUP], F8)

            # Preload order matters for startup latency: the first s-pass
            # matmuls need ah + x0 (sync queue), the first main matmuls
            # need wh chunk 0 (scalar queue, split per-chunk so MMs start
            # early instead of waiting for the whole load).
            nc.sync.dma_start(out=ah_sb[:], in_=ah_r)
            nc.sync.dma_start(out=al_sb[:], in_=al_r)
            for c in range(KP):
                nc.scalar.dma_start(out=wh_sb[:, c, :, :], in_=wh_r[:, c, :, :])
            for c in range(KP):
                nc.scalar.dma_start(out=wl_sb[:, c, :, :], in_=wl_r[:, c, :, :])
            nc.scalar.dma_start(out=btp_sb[:], in_=btp_d.ap())
            nc.gpsimd.dma_start(out=b_sb[:], in_=bias_bcast)

            for s in range(N_SUP):
                t0 = s * SUP
                tsl = slice(t0, t0 + SUP)
                xh_sb = xp.tile([P, KP, 2, SUP], F8, tag="xh")
                xl_sb = xp.tile([P, KP, 2, SUP], F8, tag="xl")
                nc.sync.dma_start(out=xh_sb[:], in_=xh_r[:, :, :, tsl])
                nc.sync.dma_start(out=xl_sb[:], in_=xl_r[:, :, :, tsl])
                # stream the mask in per-supertile so the first mask-multiply
                # isn't gated on a monolithic load
                nc.sync.dma_start(out=m_sb[:, tsl], in_=msk_d.ap()[:, tsl])

                # s.T = fused_A' @ x_hi.T for this supertile: [NR, SUP],
                # carrying the 2^7 scale from A'.
                s_ps = pss.tile([NR, SUP], F32, tag="sps")
                for c in range(KP):
                    nc.tensor.matmul(
                        s_ps[:],
                        ah_sb[:, c, :, :],
                        xh_sb[:, c, :, :],
                        start=(c == 0),
                        stop=False,
                        perf_mode=DR,
                    )
                for c in range(KP):
                    nc.tensor.matmul(
                        s_ps[:],
                        al_sb[:, c, :, :],
                        xh_sb[:, c, :, :],
                        start=False,
                        stop=(c == KP - 1),
                        perf_mode=DR,
                    )
                # sm = s * mask' quantized to fp8, written into both
                # DoubleRow j-slots (they pair with bt_hi / bt_lo).
                nc.vector.tensor_mul(sm_sb[:, 0, :], s_ps[:], m_sb[:, tsl])
                nc.vector.tensor_mul(sm_sb[:, 1, :], s_ps[:], m_sb[:, tsl])

                o_sb = op.tile([P, N_SUB, D_OUT], F32, tag="o")
                for q in range(N_SUB):
                    ts = q * SUB
                    qsl = slice(ts, ts + SUB)
                    o_ps = pso.tile([P, D_OUT], F32, tag="ops")
                    for n in range(NB):
                        nsl = slice(n * 512, (n + 1) * 512)
                        for xt_sb, wt_sb in (
                            (xh_sb, wh_sb),
                            (xl_sb, wh_sb),
                            (xh_sb, wl_sb),
                        ):
                            for c in range(KP):
                                nc.tensor.matmul(
                                    o_ps[:, nsl],
                                    xt_sb[:, c, :, qsl],
                                    wt_sb[:, c, :, nsl],
                                    start=(xt_sb is xh_sb
                                           and wt_sb is wh_sb
                                           and c == 0),
                                    stop=False,
                                    perf_mode=DR,
                                    skip_group_check=True,
                                )
                    for n in range(NB):
                        nsl = slice(n * 512, (n + 1) * 512)
                        nc.tensor.matmul(
                            o_ps[:, nsl],
                            sm_sb[:, :, qsl],
                            btp_sb[:, :, nsl],
                            start=False,
                            stop=True,
                            perf_mode=DR,
                            skip_group_check=True,
                        )
                    # out = psum * 2^-7 + bias, fused in one DVE op
                    nc.vector.scalar_tensor_tensor(
                        o_sb[:, q, :],
                        o_ps[:],
                        1.0 / SA,
                        b_sb[:],
                        mybir.AluOpType.mult,
                        mybir.AluOpType.add,
                    )
                nc.scalar.dma_start(out=out_r[s], in_=o_sb[:])

    nc.compile()
    return nc


_NC_CACHE = None


def _get_nc():
    global _NC_CACHE
    if _NC_CACHE is None:
        _NC_CACHE = build_bass()
    return _NC_CACHE


def _q8(a):
    return np.ascontiguousarray(a).astype(NPF8)


def make_in_maps(x, W, b, lora_A, lora_B, masks):
    x = np.ascontiguousarray(x, dtype=np.float32)
    W = np.ascontiguousarray(W, dtype=np.float32)
    b = np.ascontiguousarray(b, dtype=np.float32)
    lora_A = np.ascontiguousarray(lora_A, dtype=np.float32)
    lora_B = np.ascontiguousarray(lora_B, dtype=np.float32)
    masks = np.ascontiguousarray(masks, dtype=np.float32)

    x_flat = x.reshape(B * T, D_IN)
    x_hi8 = x_flat.astype(NPF8)                     # [BT, D_IN] fp8
    x_lo8 = (x_flat - x_hi8.astype(np.float32)).astype(NPF8)

    Wp = W.T.astype(np.float32) * np.float32(SA)    # [D_IN, D_OUT]
    wh8 = Wp.astype(NPF8)
    wl8 = (Wp - wh8.astype(np.float32)).astype(NPF8)
    wh8 = np.ascontiguousarray(wh8.reshape(KP, 2, P, D_OUT))
    wl8 = np.ascontiguousarray(wl8.reshape(KP, 2, P, D_OUT))

    A_flat = lora_A.reshape(NR, D_IN)
    Ap = A_flat.T.astype(np.float32) * np.float32(SA)  # [D_IN, NR]
    ah8 = Ap.astype(NPF8)
    al8 = (Ap - ah8.astype(np.float32)).astype(NPF8)
    ah8 = np.ascontiguousarray(ah8.reshape(KP, 2, P, NR))
    al8 = np.ascontiguousarray(al8.reshape(KP, 2, P, NR))

    B_flat = lora_B.transpose(1, 0, 2).reshape(D_OUT, NR)
    Bp = B_flat.T.astype(np.float32) * np.float32(SA)  # [NR, D_OUT]
    bth8 = Bp.astype(NPF8)
    btl8 = (Bp - bth8.astype(np.float32)).astype(NPF8)
    btp8 = np.ascontiguousarray(np.stack([bth8, btl8], axis=1))  # [NR,2,D_OUT]

    # mask' = mask * 2 / SA so the DVE product  s_psum * mask'  lands at
    # (2 * s * mask), exact powers of two -> fp8-exact.
    m_full = masks[..., 0].reshape(N_ADAPT, B * T)
    m_exp = np.repeat(m_full, R, axis=0) * np.float32(2.0 / SA)  # [NR, BT]
    m8 = m_exp.astype(NPF8)

    in_maps = []
    for c in range(N_CORES):
        sl = slice(c * TOK, (c + 1) * TOK)
        xh_c = np.ascontiguousarray(
            x_hi8[sl].T.reshape(KP, 2, P, TOK)
        )
        xl_c = np.ascontiguousarray(
            x_lo8[sl].T.reshape(KP, 2, P, TOK)
        )
        in_maps.append(
            {
                "xh": xh_c,
                "xl": xl_c,
                "wh": wh8,
                "wl": wl8,
                "ah": ah8,
                "al": al8,
                "btp": btp8,
                "bias": b,
                "msk": np.ascontiguousarray(m8[:, sl]),
            }
        )
    return in_maps


def kernel(x, W, b, lora_A, lora_B, masks):
    nc = _get_nc()
    in_maps = make_in_maps(x, W, b, lora_A, lora_B, masks)
    res = run_bass_kernel_spmd(nc, in_maps, core_ids=list(range(N_CORES)))
    out = np.concatenate([r["out"] for r in res.results], axis=0)
    out = out.reshape(B, T, D_OUT)
    return out


# revision 15
# speedup vs baseline: 1.5591x; 1.0673x over previous
"""Routed-LoRA linear layer (moe_routing) on 8 trn2 NeuronCores.

Math (per token t):
  out[t, :] = W @ x[t] + b + 2.0 * sum_n mask[n, t] * (B_n @ (A_n @ x[t]))

Strategy:
  - Data-parallel over B*T = 65536 tokens: 8192 tokens per core.
  - All heavy matmuls run in fp8e4m3 with perf_mode=DoubleRow (2 k-tiles
    packed per instruction, K=256 contraction per matmul, half cycle cost
    per output row vs full-rate dtypes).
  - Precision is recovered with an error-compensated split, all terms
    sharing a single 2^7 scale (on the W/A/B side) so they accumulate
    into one PSUM group:
      main : (x_hi + x_lo) @ W_hi + x_hi @ W_lo      (x_lo@W_lo dropped)
      s    : x_hi @ (A_hi + A_lo)                     (rank-64 fused A)
      delta: sm @ (B_hi + B_lo)  -- B hi/lo ride the two DoubleRow j-slots
    where *_hi = fp8(v), *_lo = fp8(v - v_hi). sm = (s * mask) quantized
    to fp8 on the fly by the Activation-engine mask-multiply.
  - The bias rides two extra contraction rows of the delta matmul
    (sm rows 64/65 memset to 1.0, btp rows 64/65 hold fp8 hi/lo of
    2^7 * b), so the epilogue is a single per-q DVE descale copy.
"""

import numpy as np
import ml_dtypes

import concourse.bass as bass
from concourse import bacc
import concourse.mybir as mybir
import concourse.tile as tile
from concourse.bass_utils import run_bass_kernel_spmd

N_CORES = 8
B, T = 8, 8192
D_IN = 1024
D_OUT = 1024
N_ADAPT, R = 4, 16
NR = N_ADAPT * R  # 64
SCALING = 32.0 / 16.0

TOK = B * T // N_CORES  # 8192 tokens per core
SUP = 512               # tokens per supertile
N_SUP = TOK // SUP      # 16
SUB = 128               # tokens per matmul M-tile
N_SUB = SUP // SUB      # 4
P = 128
KP = D_IN // 256        # 4 DoubleRow contraction chunks (256 each)
NB = D_OUT // 512       # 2 PSUM-bank column halves

SA = 2.0 ** 7           # weight-side scale so W/A/B fp8 values are ~N(0, 2.56)

F32 = mybir.dt.float32
F8 = mybir.dt.float8e4
NPF8 = ml_dtypes.float8_e4m3
DR = mybir.MatmulPerfMode.DoubleRow


def build_bass():
    nc = bacc.Bacc(
        "TRN2", target_bir_lowering=False, debug=False, num_devices=N_CORES
    )

    xh_d = nc.dram_tensor("xh", [KP, 2, P, TOK], F8, kind="ExternalInput")
    xl_d = nc.dram_tensor("xl", [KP, 2, P, TOK], F8, kind="ExternalInput")
    wh_d = nc.dram_tensor("wh", [KP, 2, P, D_OUT], F8, kind="ExternalInput")
    wl_d = nc.dram_tensor("wl", [KP, 2, P, D_OUT], F8, kind="ExternalInput")
    ah_d = nc.dram_tensor("ah", [KP, 2, P, NR], F8, kind="ExternalInput")
    btp_d = nc.dram_tensor("btp", [P, 2, D_OUT], F8, kind="ExternalInput")
    msk_d = nc.dram_tensor("msk", [NR, TOK], F8, kind="ExternalInput")
    out_d = nc.dram_tensor("out", [TOK, D_OUT], F32, kind="ExternalOutput")

    xh_r = xh_d.ap().rearrange("c j p t -> p c j t")
    xl_r = xl_d.ap().rearrange("c j p t -> p c j t")
    wh_r = wh_d.ap().rearrange("c j p n -> p c j n")
    wl_r = wl_d.ap().rearrange("c j p n -> p c j n")
    ah_r = ah_d.ap().rearrange("c j p r -> p c j r")
    out_r = out_d.ap().rearrange("(s q p) n -> s p q n", q=N_SUB, p=P)

    with tile.TileContext(nc) as tc:
        with (
            tc.tile_pool(name="const", bufs=1) as const,
            tc.tile_pool(name="xp", bufs=3) as xp,
            tc.tile_pool(name="op", bufs=2) as op,
            tc.tile_pool(name="pss", bufs=2, space="PSUM") as pss,
            tc.tile_pool(name="pso", bufs=2, space="PSUM") as pso,
        ):
            wh_sb = const.tile([P, KP, 2, D_OUT], F8)
            wl_sb = const.tile([P, KP, 2, D_OUT], F8)
            ah_sb = const.tile([P, KP, 2, NR], F8)
            btp_sb = const.tile([P, 2, D_OUT], F8)
            m_sb = const.tile([NR, TOK], F8)
            sm_sb = const.tile([P, 2, SUP], F8)

            # Rows 64..127 of sm pair with btp rows 64/65 (the fp8 hi/lo of
            # 2^7 * bias; rows 66+ are zero), adding the bias inside the
            # delta matmul for free.
            nc.vector.memset(sm_sb[NR:P, :, :], 1.0)

            # DMA ordering: all transfers serialize on one shared engine
            # pool in readiness order, and supertile 0 consumes chunks in
            # exactly this arrival order: xh0, wh0, ah, wl0, al, wh1, xl0,
            # wl1, msk0, btp, wh2, wl2, wh3, wl3. Supertile 0's own x/A/msk
            # ride the otherwise-idle sync queue; later supertiles' x/msk
            # are prefetched from the scalar queue inside the previous
            # body, where the stores' sem-waits throttle their readiness
            # so they can't cut ahead of the remaining W chunks.
            for c in range(2):
                nc.scalar.dma_start(out=wh_sb[:, c, :, :], in_=wh_r[:, c, :, :])
                nc.scalar.dma_start(out=wl_sb[:, c, :, :], in_=wl_r[:, c, :, :])
            nc.scalar.dma_start(out=btp_sb[:], in_=btp_d.ap())
            for c in range(2, KP):
                nc.scalar.dma_start(out=wh_sb[:, c, :, :], in_=wh_r[:, c, :, :])
                nc.scalar.dma_start(out=wl_sb[:, c, :, :], in_=wl_r[:, c, :, :])

            x_tiles = {}

            def prefetch(s, queue):
                t0 = s * SUP
                tsl = slice(t0, t0 + SUP)
                xh_t = xp.tile([P, KP, 2, SUP], F8, tag="xh", name=f"xh{s}")
                xl_t = xp.tile([P, KP, 2, SUP], F8, tag="xl", name=f"xl{s}")
                x_tiles[s] = (xh_t, xl_t)
                queue.dma_start(out=xh_t[:], in_=xh_r[:, :, :, tsl])
                if s == 0:
                    queue.dma_start(out=ah_sb[:], in_=ah_r)
                queue.dma_start(out=xl_t[:], in_=xl_r[:, :, :, tsl])
                queue.dma_start(out=m_sb[:, tsl], in_=msk_d.ap()[:, tsl])

            prefetch(0, nc.sync)

            for s in range(N_SUP):
                t0 = s * SUP
                tsl = slice(t0, t0 + SUP)
                xh_sb, xl_sb = x_tiles.pop(s)
                if s + 1 < N_SUP:
                    prefetch(s + 1, nc.scalar)

                s_ps = pss.tile([NR, SUP], F32, tag="sps")
                o_sb = op.tile([P, N_SUB, D_OUT], F32, tag="o")

                def s_mm(a_sb, c, start, stop):
                    nc.tensor.matmul(
                        s_ps[:],
                        a_sb[:, c, :, :],
                        xh_sb[:, c, :, :],
                        start=start,
                        stop=stop,
                        perf_mode=DR,
                    )

                def main_mm(o_ps, q, n, xt_sb, wt_sb, c, start):
                    nsl = slice(n * 512, (n + 1) * 512)
                    qsl = slice(q * SUB, q * SUB + SUB)
                    nc.tensor.matmul(
                        o_ps[:, nsl],
                        xt_sb[:, c, :, qsl],
                        wt_sb[:, c, :, nsl],
                        start=start,
                        stop=False,
                        perf_mode=DR,
                        skip_group_check=True,
                    )

                def b_mm(o_ps, q, n):
                    nsl = slice(n * 512, (n + 1) * 512)
                    qsl = slice(q * SUB, q * SUB + SUB)
                    nc.tensor.matmul(
                        o_ps[:, nsl],
                        sm_sb[:, :, qsl],
                        btp_sb[:, :, nsl],
                        start=False,
                        stop=True,
                        perf_mode=DR,
                        skip_group_check=True,
                    )

                def sm_muls():
                    # sm = s * mask' quantized to fp8, written into both
                    # DoubleRow j-slots (they pair with bt_hi / bt_lo).
                    nc.vector.tensor_mul(sm_sb[:NR, 0, :], s_ps[:], m_sb[:, tsl])
                    nc.vector.tensor_mul(sm_sb[:NR, 1, :], s_ps[:], m_sb[:, tsl])

                def finish_q(o_ps, q):
                    # out = psum * 2^-7 (bias already accumulated); per-q
                    # stores shorten the kernel tail. Descales alternate
                    # DVE / gpsimd so the DVE queue stays shallow for the
                    # sm multiplies. The very last supertile drains in
                    # fine-grained slices so the final descale->store chain
                    # after the last matmul is as short as possible.
                    for i in range(1):
                        nsl = slice(0, D_OUT)
                        if q % 2 == 0:
                            nc.vector.tensor_scalar_mul(
                                o_sb[:, q, nsl], o_ps[:, nsl], 1.0 / SA
                            )
                        else:
                            nc.scalar.activation(
                                o_sb[:, q, nsl], o_ps[:, nsl],
                                mybir.ActivationFunctionType.Copy,
                                scale=1.0 / SA,
                            )
                        nc.scalar.dma_start(
                            out=out_r[s][:, q, nsl], in_=o_sb[:, q, nsl]
                        )

                if s == 0:
                    # Supertile 0 is fed by a cold DMA pipe: consume chunks
                    # strictly in arrival order, c-outer so each W chunk
                    # unlocks matmuls the moment it lands. Round A covers
                    # q0/q1, round B (data all resident by then) q2/q3.
                    for qpair in ((0, 1), (2, 3)):
                        o_pss = {}
                        for q in qpair:
                            o_pss[q] = pso.tile(
                                [P, D_OUT], F32, tag="ops",
                                name=f"ops_s0_q{q}",
                            )

                        def pair_term(xt_sb, wt_sb, c, start=False):
                            for q in qpair:
                                for n in range(NB):
                                    main_mm(o_pss[q], q, n, xt_sb, wt_sb, c,
                                            start=start)

                        if qpair[0] == 0:
                            pair_term(xh_sb, wh_sb, 0, start=True)   # wh0
                            for c in range(KP):                      # ah
                                s_mm(ah_sb, c, start=(c == 0),
                                     stop=(c == KP - 1))
                            pair_term(xh_sb, wl_sb, 0)               # wl0
                            pair_term(xh_sb, wh_sb, 1)               # wh1
                            pair_term(xl_sb, wh_sb, 0)               # xl
                            pair_term(xl_sb, wh_sb, 1)
                            pair_term(xh_sb, wl_sb, 1)               # wl1
                            sm_muls()                                # msk0
                            pair_term(xh_sb, wh_sb, 2)               # wh2
                            pair_term(xl_sb, wh_sb, 2)
                            pair_term(xh_sb, wl_sb, 2)               # wl2
                            pair_term(xh_sb, wh_sb, 3)               # wh3
                            pair_term(xl_sb, wh_sb, 3)
                            pair_term(xh_sb, wl_sb, 3)               # wl3
                        else:
                            pair_term(xh_sb, wh_sb, 0, start=True)
                            for c in range(1, KP):
                                pair_term(xh_sb, wh_sb, c)
                            for c in range(KP):
                                pair_term(xl_sb, wh_sb, c)
                            for c in range(KP):
                                pair_term(xh_sb, wl_sb, c)
                        for q in qpair:
                            for n in range(NB):
                                b_mm(o_pss[q], q, n)                 # btp
                        for q in qpair:
                            finish_q(o_pss[q], q)
                elif s < N_SUP - 1:
                    for c in range(KP):
                        s_mm(ah_sb, c, start=(c == 0), stop=(c == KP - 1))
                    sm_muls()
                    for q in range(N_SUB):
                        o_ps = pso.tile([P, D_OUT], F32, tag="ops")
                        for xt_sb, wt_sb in (
                            (xh_sb, wh_sb),
                            (xl_sb, wh_sb),
                            (xh_sb, wl_sb),
                        ):
                            for n in range(NB):
                                for c in range(KP):
                                    main_mm(o_ps, q, n, xt_sb, wt_sb, c,
                                            start=(xt_sb is xh_sb
                                                   and wt_sb is wh_sb
                                                   and c == 0))
                        for n in range(NB):
                            b_mm(o_ps, q, n)
                        finish_q(o_ps, q)
                else:
                    # Last supertile: n-major per q with per-half epilogues
                    # so output drains while the remaining matmuls run; the
                    # very last half descales in two quarters on DVE+Pool
                    # in parallel with stores on the idle sync queue, to
                    # minimize the after-last-matmul tail.
                    for c in range(KP):
                        s_mm(ah_sb, c, start=(c == 0), stop=(c == KP - 1))
                    sm_muls()
                    for q in range(N_SUB):
                        o_ps = pso.tile([P, D_OUT], F32, tag="ops")
                        for n in range(NB):
                            for xt_sb, wt_sb in (
                                (xh_sb, wh_sb),
                                (xl_sb, wh_sb),
                                (xh_sb, wl_sb),
                            ):
                                for c in range(KP):
                                    main_mm(o_ps, q, n, xt_sb, wt_sb, c,
                                            start=(xt_sb is xh_sb
                                                   and wt_sb is wh_sb
                                                   and c == 0))
                            b_mm(o_ps, q, n)
                            nhl = slice(n * 512, (n + 1) * 512)
                            if q == N_SUB - 1 and n == NB - 1:
                                for i in range(2):
                                    qsl4 = slice(n * 512 + i * 256,
                                                 n * 512 + (i + 1) * 256)
                                    if i == 0:
                                        nc.vector.tensor_scalar_mul(
                                            o_sb[:, q, qsl4], o_ps[:, qsl4],
                                            1.0 / SA,
                                        )
                                    else:
                                        nc.scalar.activation(
                                            o_sb[:, q, qsl4], o_ps[:, qsl4],
                                            mybir.ActivationFunctionType.Copy,
                                            scale=1.0 / SA,
                                        )
                                    nc.sync.dma_start(
                                        out=out_r[s][:, q, qsl4],
                                        in_=o_sb[:, q, qsl4],
                                    )
                            else:
                                if (q + n) % 2 == 0:
                                    nc.vector.tensor_scalar_mul(
                                        o_sb[:, q, nhl], o_ps[:, nhl], 1.0 / SA
                                    )
                                else:
                                    nc.scalar.activation(
                                        o_sb[:, q, nhl], o_ps[:, nhl],
                                        mybir.ActivationFunctionType.Copy,
                                        scale=1.0 / SA,
                                    )
                                nc.scalar.dma_start(
                                    out=out_r[s][:, q, nhl],
                                    in_=o_sb[:, q, nhl],
                                )

    nc.compile()
    return nc


_NC_CACHE = None


def _get_nc():
    global _NC_CACHE
    if _NC_CACHE is None:
        _NC_CACHE = build_bass()
    return _NC_CACHE


def make_in_maps(x, W, b, lora_A, lora_B, masks):
    x = np.ascontiguousarray(x, dtype=np.float32)
    W = np.ascontiguousarray(W, dtype=np.float32)
    b = np.ascontiguousarray(b, dtype=np.float32)
    lora_A = np.ascontiguousarray(lora_A, dtype=np.float32)
    lora_B = np.ascontiguousarray(lora_B, dtype=np.float32)
    masks = np.ascontiguousarray(masks, dtype=np.float32)

    x_flat = x.reshape(B * T, D_IN)
    x_hi8 = x_flat.astype(NPF8)                     # [BT, D_IN] fp8
    x_lo8 = (x_flat - x_hi8.astype(np.float32)).astype(NPF8)

    Wp = W.T.astype(np.float32) * np.float32(SA)    # [D_IN, D_OUT]
    wh8 = Wp.astype(NPF8)
    wl8 = (Wp - wh8.astype(np.float32)).astype(NPF8)
    wh8 = np.ascontiguousarray(wh8.reshape(KP, 2, P, D_OUT))
    wl8 = np.ascontiguousarray(wl8.reshape(KP, 2, P, D_OUT))

    A_flat = lora_A.reshape(NR, D_IN)
    Ap = A_flat.T.astype(np.float32) * np.float32(SA)  # [D_IN, NR]
    ah8 = np.ascontiguousarray(Ap.astype(NPF8).reshape(KP, 2, P, NR))

    B_flat = lora_B.transpose(1, 0, 2).reshape(D_OUT, NR)
    Bp = B_flat.T.astype(np.float32) * np.float32(SA)  # [NR, D_OUT]
    bth8 = Bp.astype(NPF8)
    btl8 = (Bp - bth8.astype(np.float32)).astype(NPF8)
    # btp rows 0..63: j0 = B_hi, j1 = B_lo. Rows 64/65: fp8 hi/lo of the
    # scaled bias, paired with sm rows memset to 1.0. Rows 66+: zero.
    btp8 = np.zeros((P, 2, D_OUT), dtype=NPF8)
    btp8[:NR, 0, :] = bth8
    btp8[:NR, 1, :] = btl8
    bp = b.astype(np.float32) * np.float32(SA)
    bh8 = bp.astype(NPF8)
    bl8 = (bp - bh8.astype(np.float32)).astype(NPF8)
    btp8[NR, 0, :] = bh8
    btp8[NR + 1, 0, :] = bl8
    btp8 = np.ascontiguousarray(btp8)

    # mask' = mask * 2 / SA so the product  s_psum * mask'  lands at
    # (2 * s * mask), exact powers of two -> fp8-exact.
    m_full = masks[..., 0].reshape(N_ADAPT, B * T)
    m_exp = np.repeat(m_full, R, axis=0) * np.float32(2.0 / SA)  # [NR, BT]
    m8 = m_exp.astype(NPF8)

    in_maps = []
    for c in range(N_CORES):
        sl = slice(c * TOK, (c + 1) * TOK)
        xh_c = np.ascontiguousarray(x_hi8[sl].T.reshape(KP, 2, P, TOK))
        xl_c = np.ascontiguousarray(x_lo8[sl].T.reshape(KP, 2, P, TOK))
        in_maps.append(
            {
                "xh": xh_c,
                "xl": xl_c,
                "wh": wh8,
                "wl": wl8,
                "ah": ah8,
                "btp": btp8,
                "msk": np.ascontiguousarray(m8[:, sl]),
            }
        )
    return in_maps


def kernel(x, W, b, lora_A, lora_B, masks):
    nc = _get_nc()
    in_maps = make_in_maps(x, W, b, lora_A, lora_B, masks)
    res = run_bass_kernel_spmd(nc, in_maps, core_ids=list(range(N_CORES)))
    out = np.concatenate([r["out"] for r in res.results], axis=0)
    out = out.reshape(B, T, D_OUT)
    return out


# revision 20
# speedup vs baseline: 1.5864x; 1.0175x over previous
"""Routed-LoRA linear layer (moe_routing) on 8 trn2 NeuronCores.

Math (per token t):
  out[t, :] = W @ x[t] + b + 2.0 * sum_n mask[n, t] * (B_n @ (A_n @ x[t]))

Strategy:
  - Data-parallel over B*T = 65536 tokens: 8192 tokens per core.
  - All heavy matmuls run in fp8e4m3 with perf_mode=DoubleRow (2 k-tiles
    packed per instruction, K=256 contraction per matmul, half cycle cost
    per output row vs full-rate dtypes).
  - Precision is recovered with an error-compensated split, all terms
    sharing a single 2^7 scale (on the W/A/B side) so they accumulate
    into one PSUM group:
      main : (x_hi + x_lo) @ W_hi + x_hi @ W_lo      (x_lo@W_lo dropped)
      s    : x_hi @ (A_hi + A_lo)                     (rank-64 fused A)
      delta: sm @ (B_hi + B_lo)  -- B hi/lo ride the two DoubleRow j-slots
    where *_hi = fp8(v), *_lo = fp8(v - v_hi). sm = (s * mask) quantized
    to fp8 on the fly by the Activation-engine mask-multiply.
  - The bias rides two extra contraction rows of the delta matmul
    (sm rows 64/65 memset to 1.0, btp rows 64/65 hold fp8 hi/lo of
    2^7 * b), so the epilogue is a single per-q DVE descale copy.
"""

import numpy as np
import ml_dtypes

import concourse.bass as bass
from concourse import bacc
import concourse.mybir as mybir
import concourse.tile as tile
from concourse.bass_utils import run_bass_kernel_spmd

N_CORES = 8
B, T = 8, 8192
D_IN = 1024
D_OUT = 1024
N_ADAPT, R = 4, 16
NR = N_ADAPT * R  # 64
SCALING = 32.0 / 16.0

TOK = B * T // N_CORES  # 8192 tokens per core
SUP = 512               # tokens per supertile
N_SUP = TOK // SUP      # 16
SUB = 128               # tokens per matmul M-tile
N_SUB = SUP // SUB      # 4
P = 128
KP = D_IN // 256        # 4 DoubleRow contraction chunks (256 each)
NB = D_OUT // 512       # 2 PSUM-bank column halves

SA = 2.0 ** 7           # weight-side scale so W/A/B fp8 values are ~N(0, 2.56)

F32 = mybir.dt.float32
F8 = mybir.dt.float8e4
NPF8 = ml_dtypes.float8_e4m3
DR = mybir.MatmulPerfMode.DoubleRow


def build_bass():
    nc = bacc.Bacc(
        "TRN2", target_bir_lowering=False, debug=False, num_devices=N_CORES
    )

    xh_d = nc.dram_tensor("xh", [KP, 2, P, TOK], F8, kind="ExternalInput")
    xl_d = nc.dram_tensor("xl", [KP, 2, P, TOK], F8, kind="ExternalInput")
    wh_d = nc.dram_tensor("wh", [KP, 2, P, D_OUT], F8, kind="ExternalInput")
    wl_d = nc.dram_tensor("wl", [KP, 2, P, D_OUT], F8, kind="ExternalInput")
    ah_d = nc.dram_tensor("ah", [KP, 2, P, NR], F8, kind="ExternalInput")
    btp_d = nc.dram_tensor("btp", [P, 2, D_OUT], F8, kind="ExternalInput")
    msk_d = nc.dram_tensor("msk", [NR, TOK], F8, kind="ExternalInput")
    out_d = nc.dram_tensor("out", [TOK, D_OUT], F32, kind="ExternalOutput")

    xh_r = xh_d.ap().rearrange("c j p t -> p c j t")
    xl_r = xl_d.ap().rearrange("c j p t -> p c j t")
    wh_r = wh_d.ap().rearrange("c j p n -> p c j n")
    wl_r = wl_d.ap().rearrange("c j p n -> p c j n")
    ah_r = ah_d.ap().rearrange("c j p r -> p c j r")
    out_r = out_d.ap().rearrange("(s q p) n -> s p q n", q=N_SUB, p=P)

    with tile.TileContext(nc) as tc:
        with (
            tc.tile_pool(name="const", bufs=1) as const,
            tc.tile_pool(name="xp", bufs=3) as xp,
            tc.tile_pool(name="op", bufs=2) as op,
            tc.tile_pool(name="pss", bufs=2, space="PSUM") as pss,
            tc.tile_pool(name="pso", bufs=3, space="PSUM") as pso,
        ):
            wh_sb = const.tile([P, KP, 2, D_OUT], F8)
            wl_sb = const.tile([P, KP, 2, D_OUT], F8)
            ah_sb = const.tile([P, KP, 2, NR], F8)
            btp_sb = const.tile([P, 2, D_OUT], F8)
            m_sb = const.tile([NR, TOK], F8)
            sm_sb = const.tile([P, 2, SUP], F8)

            # Rows 64..127 of sm pair with btp rows 64/65 (the fp8 hi/lo of
            # 2^7 * bias; rows 66+ are zero), adding the bias inside the
            # delta matmul for free.
            nc.vector.memset(sm_sb[NR:P, :, :], 1.0)

            # DMA ordering: all transfers serialize on one shared engine
            # pool in readiness order, and supertile 0 consumes chunks in
            # exactly this arrival order: xh0, wh0, ah, wl0, al, wh1, xl0,
            # wl1, msk0, btp, wh2, wl2, wh3, wl3. Supertile 0's own x/A/msk
            # ride the otherwise-idle sync queue; later supertiles' x/msk
            # are prefetched from the scalar queue inside the previous
            # body, where the stores' sem-waits throttle their readiness
            # so they can't cut ahead of the remaining W chunks.
            for c in range(2):
                nc.scalar.dma_start(out=wh_sb[:, c, :, :], in_=wh_r[:, c, :, :])
                nc.scalar.dma_start(out=wl_sb[:, c, :, :], in_=wl_r[:, c, :, :])
            nc.scalar.dma_start(out=btp_sb[:], in_=btp_d.ap())
            for c in range(2, KP):
                nc.scalar.dma_start(out=wh_sb[:, c, :, :], in_=wh_r[:, c, :, :])
                nc.scalar.dma_start(out=wl_sb[:, c, :, :], in_=wl_r[:, c, :, :])

            x_tiles = {}

            def prefetch(s, queue):
                t0 = s * SUP
                tsl = slice(t0, t0 + SUP)
                xh_t = xp.tile([P, KP, 2, SUP], F8, tag="xh", name=f"xh{s}")
                xl_t = xp.tile([P, KP, 2, SUP], F8, tag="xl", name=f"xl{s}")
                x_tiles[s] = (xh_t, xl_t)
                queue.dma_start(out=xh_t[:], in_=xh_r[:, :, :, tsl])
                if s == 0:
                    queue.dma_start(out=ah_sb[:], in_=ah_r)
                queue.dma_start(out=xl_t[:], in_=xl_r[:, :, :, tsl])
                queue.dma_start(out=m_sb[:, tsl], in_=msk_d.ap()[:, tsl])

            prefetch(0, nc.sync)

            for s in range(N_SUP):
                t0 = s * SUP
                tsl = slice(t0, t0 + SUP)
                xh_sb, xl_sb = x_tiles.pop(s)
                if s + 1 < N_SUP:
                    prefetch(s + 1, nc.scalar)

                s_ps = pss.tile([NR, SUP], F32, tag="sps")
                o_sb = op.tile([P, N_SUB, D_OUT], F32, tag="o")

                def s_mm(a_sb, c, start, stop):
                    nc.tensor.matmul(
                        s_ps[:],
                        a_sb[:, c, :, :],
                        xh_sb[:, c, :, :],
                        start=start,
                        stop=stop,
                        perf_mode=DR,
                    )

                def main_mm(o_ps, q, n, xt_sb, wt_sb, c, start):
                    nsl = slice(n * 512, (n + 1) * 512)
                    qsl = slice(q * SUB, q * SUB + SUB)
                    nc.tensor.matmul(
                        o_ps[:, nsl],
                        xt_sb[:, c, :, qsl],
                        wt_sb[:, c, :, nsl],
                        start=start,
                        stop=False,
                        perf_mode=DR,
                        skip_group_check=True,
                    )

                def b_mm(o_ps, q, n):
                    nsl = slice(n * 512, (n + 1) * 512)
                    qsl = slice(q * SUB, q * SUB + SUB)
                    nc.tensor.matmul(
                        o_ps[:, nsl],
                        sm_sb[:, :, qsl],
                        btp_sb[:, :, nsl],
                        start=False,
                        stop=True,
                        perf_mode=DR,
                        skip_group_check=True,
                    )

                def sm_muls():
                    # sm = s * mask' quantized to fp8, written into both
                    # DoubleRow j-slots (they pair with bt_hi / bt_lo).
                    nc.vector.tensor_mul(sm_sb[:NR, 0, :], s_ps[:], m_sb[:, tsl])
                    nc.vector.tensor_mul(sm_sb[:NR, 1, :], s_ps[:], m_sb[:, tsl])

                def finish_q(o_ps, q):
                    # out = psum * 2^-7 (bias already accumulated); per-q
                    # stores shorten the kernel tail. Descales alternate
                    # DVE / gpsimd so the DVE queue stays shallow for the
                    # sm multiplies. The very last supertile drains in
                    # fine-grained slices so the final descale->store chain
                    # after the last matmul is as short as possible.
                    for i in range(1):
                        nsl = slice(0, D_OUT)
                        if q % 2 == 0:
                            nc.vector.tensor_scalar_mul(
                                o_sb[:, q, nsl], o_ps[:, nsl], 1.0 / SA
                            )
                        else:
                            nc.scalar.activation(
                                o_sb[:, q, nsl], o_ps[:, nsl],
                                mybir.ActivationFunctionType.Copy,
                                scale=1.0 / SA,
                            )
                        nc.gpsimd.dma_start(
                            out=out_r[s][:, q, nsl], in_=o_sb[:, q, nsl]
                        )

                if s == 0:
                    # Supertile 0 is fed by a cold DMA pipe: consume chunks
                    # strictly in arrival order (xh0, wh0, ah, wl0, xl0,
                    # wh1, wl1, msk0, btp, wh2, wl2, wh3, wl3), c-outer
                    # over a q-triple so each W chunk unlocks ~640ns of
                    # matmuls the moment it lands (chunks arrive every
                    # ~730ns). q3 runs as round B with all data resident.
                    for qgrp in ((0, 1), (2, 3)):
                        o_pss = {}
                        for q in qgrp:
                            o_pss[q] = pso.tile(
                                [P, D_OUT], F32, tag="ops",
                                name=f"ops_s0_q{q}",
                            )

                        def grp_term(xt_sb, wt_sb, c, start=False):
                            for q in qgrp:
                                for n in range(NB):
                                    main_mm(o_pss[q], q, n, xt_sb, wt_sb, c,
                                            start=start)

                        if qgrp[0] == 0:
                            grp_term(xh_sb, wh_sb, 0, start=True)    # wh0
                            for c in range(KP):                      # ah
                                s_mm(ah_sb, c, start=(c == 0),
                                     stop=(c == KP - 1))
                            grp_term(xh_sb, wl_sb, 0)                # wl0
                            grp_term(xl_sb, wh_sb, 0)                # xl0
                            grp_term(xh_sb, wh_sb, 1)                # wh1
                            grp_term(xl_sb, wh_sb, 1)
                            grp_term(xh_sb, wl_sb, 1)                # wl1
                            sm_muls()                                # msk0
                            grp_term(xh_sb, wh_sb, 2)                # wh2
                            grp_term(xl_sb, wh_sb, 2)
                            grp_term(xh_sb, wl_sb, 2)                # wl2
                            grp_term(xh_sb, wh_sb, 3)                # wh3
                            grp_term(xl_sb, wh_sb, 3)
                            grp_term(xh_sb, wl_sb, 3)                # wl3
                        else:
                            grp_term(xh_sb, wh_sb, 0, start=True)
                            for c in range(1, KP):
                                grp_term(xh_sb, wh_sb, c)
                            for c in range(KP):
                                grp_term(xl_sb, wh_sb, c)
                            for c in range(KP):
                                grp_term(xh_sb, wl_sb, c)
                        for q in qgrp:
                            for n in range(NB):
                                b_mm(o_pss[q], q, n)                 # btp
                        for q in qgrp:
                            finish_q(o_pss[q], q)
                elif s < N_SUP - 1:
                    for c in range(KP):
                        s_mm(ah_sb, c, start=(c == 0), stop=(c == KP - 1))
                    sm_muls()
                    for q in range(N_SUB):
                        o_ps = pso.tile([P, D_OUT], F32, tag="ops")
                        for xt_sb, wt_sb in (
                            (xh_sb, wh_sb),
                            (xl_sb, wh_sb),
                            (xh_sb, wl_sb),
                        ):
                            for n in range(NB):
                                for c in range(KP):
                                    main_mm(o_ps, q, n, xt_sb, wt_sb, c,
                                            start=(xt_sb is xh_sb
                                                   and wt_sb is wh_sb
                                                   and c == 0))
                        for n in range(NB):
                            b_mm(o_ps, q, n)
                        finish_q(o_ps, q)
                else:
                    # Last supertile: n-major per q with per-half epilogues
                    # so output drains while the remaining matmuls run; the
                    # very last half descales in two quarters on DVE+Pool
                    # in parallel with stores on the idle sync queue, to
                    # minimize the after-last-matmul tail.
                    for c in range(KP):
                        s_mm(ah_sb, c, start=(c == 0), stop=(c == KP - 1))
                    sm_muls()
                    for q in range(N_SUB):
                        o_ps = pso.tile([P, D_OUT], F32, tag="ops")
                        if q < N_SUB - 1:
                            # steady order; only the last q runs n-major
                            for xt_sb, wt_sb in (
                                (xh_sb, wh_sb),
                                (xl_sb, wh_sb),
                                (xh_sb, wl_sb),
                            ):
                                for n in range(NB):
                                    for c in range(KP):
                                        main_mm(o_ps, q, n, xt_sb, wt_sb, c,
                                                start=(xt_sb is xh_sb
                                                       and wt_sb is wh_sb
                                                       and c == 0))
                        for n in range(NB):
                            if q == N_SUB - 1:
                                for xt_sb, wt_sb in (
                                    (xh_sb, wh_sb),
                                    (xl_sb, wh_sb),
                                    (xh_sb, wl_sb),
                                ):
                                    for c in range(KP):
                                        main_mm(o_ps, q, n, xt_sb, wt_sb, c,
                                                start=(xt_sb is xh_sb
                                                       and wt_sb is wh_sb
                                                       and c == 0))
                            b_mm(o_ps, q, n)
                            nhl = slice(n * 512, (n + 1) * 512)
                            if q == N_SUB - 1 and n == NB - 1:
                                for i in range(2):
                                    qsl4 = slice(n * 512 + i * 256,
                                                 n * 512 + (i + 1) * 256)
                                    if i == 0:
                                        nc.vector.tensor_scalar_mul(
                                            o_sb[:, q, qsl4], o_ps[:, qsl4],
                                            1.0 / SA,
                                        )
                                    else:
                                        nc.scalar.activation(
                                            o_sb[:, q, qsl4], o_ps[:, qsl4],
                                            mybir.ActivationFunctionType.Copy,
                                            scale=1.0 / SA,
                                        )
                                    nc.sync.dma_start(
                                        out=out_r[s][:, q, qsl4],
                                        in_=o_sb[:, q, qsl4],
                                    )
                            else:
                                if (q + n) % 2 == 0:
                                    nc.vector.tensor_scalar_mul(
                                        o_sb[:, q, nhl], o_ps[:, nhl], 1.0 / SA
                                    )
                                else:
                                    nc.scalar.activation(
                                        o_sb[:, q, nhl], o_ps[:, nhl],
                                        mybir.ActivationFunctionType.Copy,
                                        scale=1.0 / SA,
                                    )
                                nc.gpsimd.dma_start(
                                    out=out_r[s][:, q, nhl],
                                    in_=o_sb[:, q, nhl],
                                )

    nc.compile()
    return nc


_NC_CACHE = None


def _get_nc():
    global _NC_CACHE
    if _NC_CACHE is None:
        _NC_CACHE = build_bass()
    return _NC_CACHE


def make_in_maps(x, W, b, lora_A, lora_B, masks):
    x = np.ascontiguousarray(x, dtype=np.float32)
    W = np.ascontiguousarray(W, dtype=np.float32)
    b = np.ascontiguousarray(b, dtype=np.float32)
    lora_A = np.ascontiguousarray(lora_A, dtype=np.float32)
    lora_B = np.ascontiguousarray(lora_B, dtype=np.float32)
    masks = np.ascontiguousarray(masks, dtype=np.float32)

    x_flat = x.reshape(B * T, D_IN)
    x_hi8 = x_flat.astype(NPF8)                     # [BT, D_IN] fp8
    x_lo8 = (x_flat - x_hi8.astype(np.float32)).astype(NPF8)

    Wp = W.T.astype(np.float32) * np.float32(SA)    # [D_IN, D_OUT]
    wh8 = Wp.astype(NPF8)
    wl8 = (Wp - wh8.astype(np.float32)).astype(NPF8)
    wh8 = np.ascontiguousarray(wh8.reshape(KP, 2, P, D_OUT))
    wl8 = np.ascontiguousarray(wl8.reshape(KP, 2, P, D_OUT))

    A_flat = lora_A.reshape(NR, D_IN)
    Ap = A_flat.T.astype(np.float32) * np.float32(SA)  # [D_IN, NR]
    ah8 = np.ascontiguousarray(Ap.astype(NPF8).reshape(KP, 2, P, NR))

    B_flat = lora_B.transpose(1, 0, 2).reshape(D_OUT, NR)
    Bp = B_flat.T.astype(np.float32) * np.float32(SA)  # [NR, D_OUT]
    bth8 = Bp.astype(NPF8)
    btl8 = (Bp - bth8.astype(np.float32)).astype(NPF8)
    # btp rows 0..63: j0 = B_hi, j1 = B_lo. Rows 64/65: fp8 hi/lo of the
    # scaled bias, paired with sm rows memset to 1.0. Rows 66+: zero.
    btp8 = np.zeros((P, 2, D_OUT), dtype=NPF8)
    btp8[:NR, 0, :] = bth8
    btp8[:NR, 1, :] = btl8
    bp = b.astype(np.float32) * np.float32(SA)
    bh8 = bp.astype(NPF8)
    bl8 = (bp - bh8.astype(np.float32)).astype(NPF8)
    btp8[NR, 0, :] = bh8
    btp8[NR + 1, 0, :] = bl8
    btp8 = np.ascontiguousarray(btp8)

    # mask' = mask * 2 / SA so the product  s_psum * mask'  lands at
    # (2 * s * mask), exact powers of two -> fp8-exact.
    m_full = masks[..., 0].reshape(N_ADAPT, B * T)
    m_exp = np.repeat(m_full, R, axis=0) * np.float32(2.0 / SA)  # [NR, BT]
    m8 = m_exp.astype(NPF8)

    in_maps = []
    for c in range(N_CORES):
        sl = slice(c * TOK, (c + 1) * TOK)
        xh_c = np.ascontiguousarray(x_hi8[sl].T.reshape(KP, 2, P, TOK))
        xl_c = np.ascontiguousarray(x_lo8[sl].T.reshape(KP, 2, P, TOK))
        in_maps.append(
            {
                "xh": xh_c,
                "xl": xl_c,
                "wh": wh8,
                "wl": wl8,
                "ah": ah8,
                "btp": btp8,
                "msk": np.ascontiguousarray(m8[:, sl]),
            }
        )
    return in_maps


def kernel(x, W, b, lora_A, lora_B, masks):
    nc = _get_nc()
    in_maps = make_in_maps(x, W, b, lora_A, lora_B, masks)
    res = run_bass_kernel_spmd(nc, in_maps, core_ids=list(range(N_CORES)))
    out = np.concatenate([r["out"] for r in res.results], axis=0)
    out = out.reshape(B, T, D_OUT)
    return out


# revision 28
# speedup vs baseline: 1.5883x; 1.0012x over previous
"""Routed-LoRA linear layer (moe_routing) on 8 trn2 NeuronCores.

Math (per token t):
  out[t, :] = W @ x[t] + b + 2.0 * sum_n mask[n, t] * (B_n @ (A_n @ x[t]))

Strategy:
  - Data-parallel over B*T = 65536 tokens: 8192 tokens per core.
  - All heavy matmuls run in fp8e4m3 with perf_mode=DoubleRow (2 k-tiles
    packed per instruction, K=256 contraction per matmul, half cycle cost
    per output row vs full-rate dtypes).
  - Precision is recovered with an error-compensated split, all terms
    sharing a single 2^7 scale (on the W/A/B side) so they accumulate
    into one PSUM group:
      main : (x_hi + x_lo) @ W_hi + x_hi @ W_lo      (x_lo@W_lo dropped)
      s    : x_hi @ (A_hi + A_lo)                     (rank-64 fused A)
      delta: sm @ (B_hi + B_lo)  -- B hi/lo ride the two DoubleRow j-slots
    where *_hi = fp8(v), *_lo = fp8(v - v_hi). sm = (s * mask) quantized
    to fp8 on the fly by the Activation-engine mask-multiply.
  - The bias rides two extra contraction rows of the delta matmul
    (sm rows 64/65 memset to 1.0, btp rows 64/65 hold fp8 hi/lo of
    2^7 * b), so the epilogue is a single per-q DVE descale copy.
"""

import numpy as np
import ml_dtypes

import concourse.bass as bass
from concourse import bacc
import concourse.mybir as mybir
import concourse.tile as tile
from concourse.bass_utils import run_bass_kernel_spmd

N_CORES = 8
B, T = 8, 8192
D_IN = 1024
D_OUT = 1024
N_ADAPT, R = 4, 16
NR = N_ADAPT * R  # 64
SCALING = 32.0 / 16.0

TOK = B * T // N_CORES  # 8192 tokens per core
SUP = 512               # tokens per supertile
N_SUP = TOK // SUP      # 16
SUB = 128               # tokens per matmul M-tile
N_SUB = SUP // SUB      # 4
P = 128
KP = D_IN // 256        # 4 DoubleRow contraction chunks (256 each)
NB = D_OUT // 512       # 2 PSUM-bank column halves

SA = 2.0 ** 7           # weight-side scale so W/A/B fp8 values are ~N(0, 2.56)

F32 = mybir.dt.float32
F8 = mybir.dt.float8e4
NPF8 = ml_dtypes.float8_e4m3
DR = mybir.MatmulPerfMode.DoubleRow


def build_bass():
    nc = bacc.Bacc(
        "TRN2", target_bir_lowering=False, debug=False, num_devices=N_CORES
    )

    xh_d = nc.dram_tensor("xh", [KP, 2, P, TOK], F8, kind="ExternalInput")
    xl_d = nc.dram_tensor("xl", [KP, 2, P, TOK], F8, kind="ExternalInput")
    wh_d = nc.dram_tensor("wh", [KP, 2, P, D_OUT], F8, kind="ExternalInput")
    wl_d = nc.dram_tensor("wl", [KP, 2, P, D_OUT], F8, kind="ExternalInput")
    ah_d = nc.dram_tensor("ah", [KP, 2, P, NR], F8, kind="ExternalInput")
    btp_d = nc.dram_tensor("btp", [P, 2, D_OUT], F8, kind="ExternalInput")
    msk_d = nc.dram_tensor("msk", [NR, TOK], F8, kind="ExternalInput")
    out_d = nc.dram_tensor("out", [TOK, D_OUT], F32, kind="ExternalOutput")

    xh_r = xh_d.ap().rearrange("c j p t -> p c j t")
    xl_r = xl_d.ap().rearrange("c j p t -> p c j t")
    wh_r = wh_d.ap().rearrange("c j p n -> p c j n")
    wl_r = wl_d.ap().rearrange("c j p n -> p c j n")
    ah_r = ah_d.ap().rearrange("c j p r -> p c j r")
    out_r = out_d.ap().rearrange("(s q p) n -> s p q n", q=N_SUB, p=P)

    with tile.TileContext(nc) as tc:
        with (
            tc.tile_pool(name="const", bufs=1) as const,
            tc.tile_pool(name="xp", bufs=3) as xp,
            tc.tile_pool(name="op", bufs=2) as op,
            tc.tile_pool(name="pss", bufs=2, space="PSUM") as pss,
            tc.tile_pool(name="pso", bufs=3, space="PSUM") as pso,
        ):
            wh_sb = const.tile([P, KP, 2, D_OUT], F8)
            wl_sb = const.tile([P, KP, 2, D_OUT], F8)
            ah_sb = const.tile([P, KP, 2, NR], F8)
            btp_sb = const.tile([P, 2, D_OUT], F8)
            m_sb = const.tile([NR, TOK], F8)
            sm_sb = const.tile([P, 2, SUP], F8)

            # Rows 64..127 of sm pair with btp rows 64/65 (the fp8 hi/lo of
            # 2^7 * bias; rows 66+ are zero), adding the bias inside the
            # delta matmul for free.
            nc.vector.memset(sm_sb[NR:P, :, :], 1.0)

            # DMA ordering: all transfers serialize on one shared engine
            # pool in readiness order, and supertile 0 consumes chunks in
            # exactly this arrival order: xh0, wh0, ah, wl0, al, wh1, xl0,
            # wl1, msk0, btp, wh2, wl2, wh3, wl3. Supertile 0's own x/A/msk
            # ride the otherwise-idle sync queue; later supertiles' x/msk
            # are prefetched from the scalar queue inside the previous
            # body, where the stores' sem-waits throttle their readiness
            # so they can't cut ahead of the remaining W chunks.
            for c in range(2):
                nc.scalar.dma_start(out=wh_sb[:, c, :, :], in_=wh_r[:, c, :, :])
                nc.scalar.dma_start(out=wl_sb[:, c, :, :], in_=wl_r[:, c, :, :])
            nc.scalar.dma_start(out=btp_sb[:], in_=btp_d.ap())
            for c in range(2, KP):
                nc.scalar.dma_start(out=wh_sb[:, c, :, :], in_=wh_r[:, c, :, :])
                nc.scalar.dma_start(out=wl_sb[:, c, :, :], in_=wl_r[:, c, :, :])

            x_tiles = {}

            def prefetch(s, queue):
                t0 = s * SUP
                tsl = slice(t0, t0 + SUP)
                xh_t = xp.tile([P, KP, 2, SUP], F8, tag="xh", name=f"xh{s}")
                xl_t = xp.tile([P, KP, 2, SUP], F8, tag="xl", name=f"xl{s}")
                x_tiles[s] = (xh_t, xl_t)
                if s == 0:
                    # split the cold-start x loads so the first matmul's
                    # chunk (k-pair 0) lands one pool-slot earlier
                    queue.dma_start(out=xh_t[:, :2], in_=xh_r[:, :2, :, tsl])
                    queue.dma_start(out=xh_t[:, 2:], in_=xh_r[:, 2:, :, tsl])
                    queue.dma_start(out=ah_sb[:], in_=ah_r)
                    queue.dma_start(out=xl_t[:, :2], in_=xl_r[:, :2, :, tsl])
                    queue.dma_start(out=xl_t[:, 2:], in_=xl_r[:, 2:, :, tsl])
                else:
                    queue.dma_start(out=xh_t[:], in_=xh_r[:, :, :, tsl])
                    queue.dma_start(out=xl_t[:], in_=xl_r[:, :, :, tsl])
                queue.dma_start(out=m_sb[:, tsl], in_=msk_d.ap()[:, tsl])

            prefetch(0, nc.sync)

            for s in range(N_SUP):
                t0 = s * SUP
                tsl = slice(t0, t0 + SUP)
                xh_sb, xl_sb = x_tiles.pop(s)
                if s + 1 < N_SUP:
                    prefetch(s + 1, nc.scalar)

                s_ps = pss.tile([NR, SUP], F32, tag="sps")
                o_sb = op.tile([P, N_SUB, D_OUT], F32, tag="o")

                def s_mm(a_sb, c, start, stop):
                    nc.tensor.matmul(
                        s_ps[:],
                        a_sb[:, c, :, :],
                        xh_sb[:, c, :, :],
                        start=start,
                        stop=stop,
                        perf_mode=DR,
                    )

                def main_mm(o_ps, q, n, xt_sb, wt_sb, c, start):
                    nsl = slice(n * 512, (n + 1) * 512)
                    qsl = slice(q * SUB, q * SUB + SUB)
                    nc.tensor.matmul(
                        o_ps[:, nsl],
                        xt_sb[:, c, :, qsl],
                        wt_sb[:, c, :, nsl],
                        start=start,
                        stop=False,
                        perf_mode=DR,
                        skip_group_check=True,
                    )

                def b_mm(o_ps, q, n):
                    nsl = slice(n * 512, (n + 1) * 512)
                    qsl = slice(q * SUB, q * SUB + SUB)
                    nc.tensor.matmul(
                        o_ps[:, nsl],
                        sm_sb[:, :, qsl],
                        btp_sb[:, :, nsl],
                        start=False,
                        stop=True,
                        perf_mode=DR,
                        skip_group_check=True,
                    )

                def sm_muls():
                    # sm = s * mask' quantized to fp8, written into both
                    # DoubleRow j-slots (they pair with bt_hi / bt_lo).
                    nc.vector.tensor_mul(sm_sb[:NR, 0, :], s_ps[:], m_sb[:, tsl])
                    nc.vector.tensor_mul(sm_sb[:NR, 1, :], s_ps[:], m_sb[:, tsl])

                def finish_q(o_ps, q):
                    # out = psum * 2^-7 (bias already accumulated); per-q
                    # stores shorten the kernel tail. Descales alternate
                    # DVE / gpsimd so the DVE queue stays shallow for the
                    # sm multiplies. The very last supertile drains in
                    # fine-grained slices so the final descale->store chain
                    # after the last matmul is as short as possible.
                    for i in range(1):
                        nsl = slice(0, D_OUT)
                        if q % 2 == 0:
                            nc.vector.tensor_scalar_mul(
                                o_sb[:, q, nsl], o_ps[:, nsl], 1.0 / SA
                            )
                        else:
                            nc.scalar.activation(
                                o_sb[:, q, nsl], o_ps[:, nsl],
                                mybir.ActivationFunctionType.Copy,
                                scale=1.0 / SA,
                            )
                        nc.gpsimd.dma_start(
                            out=out_r[s][:, q, nsl], in_=o_sb[:, q, nsl]
                        )

                if s == 0:
                    # Supertile 0 is fed by a cold DMA pipe: consume chunks
                    # strictly in arrival order (xh0, wh0, ah, wl0, xl0,
                    # wh1, wl1, msk0, btp, wh2, wl2, wh3, wl3), c-outer
                    # over a q-triple so each W chunk unlocks ~640ns of
                    # matmuls the moment it lands (chunks arrive every
                    # ~730ns). q3 runs as round B with all data resident.
                    for qgrp in ((0, 1), (2, 3)):
                        o_pss = {}
                        for q in qgrp:
                            o_pss[q] = pso.tile(
                                [P, D_OUT], F32, tag="ops",
                                name=f"ops_s0_q{q}",
                            )

                        def grp_term(xt_sb, wt_sb, c, start=False):
                            for q in qgrp:
                                for n in range(NB):
                                    main_mm(o_pss[q], q, n, xt_sb, wt_sb, c,
                                            start=start)

                        if qgrp[0] == 0:
                            grp_term(xh_sb, wh_sb, 0, start=True)    # wh0
                            for c in range(KP):                      # ah
                                s_mm(ah_sb, c, start=(c == 0),
                                     stop=(c == KP - 1))
                            grp_term(xh_sb, wl_sb, 0)                # wl0
                            grp_term(xl_sb, wh_sb, 0)                # xl0
                            grp_term(xh_sb, wh_sb, 1)                # wh1
                            grp_term(xl_sb, wh_sb, 1)
                            grp_term(xh_sb, wl_sb, 1)                # wl1
                            sm_muls()                                # msk0
                            grp_term(xh_sb, wh_sb, 2)                # wh2
                            grp_term(xl_sb, wh_sb, 2)
                            grp_term(xh_sb, wl_sb, 2)                # wl2
                            grp_term(xh_sb, wh_sb, 3)                # wh3
                            grp_term(xl_sb, wh_sb, 3)
                            grp_term(xh_sb, wl_sb, 3)                # wl3
                        else:
                            grp_term(xh_sb, wh_sb, 0, start=True)
                            for c in range(1, KP):
                                grp_term(xh_sb, wh_sb, c)
                            for c in range(KP):
                                grp_term(xl_sb, wh_sb, c)
                            for c in range(KP):
                                grp_term(xh_sb, wl_sb, c)
                        for q in qgrp:
                            for n in range(NB):
                                b_mm(o_pss[q], q, n)                 # btp
                        for q in qgrp:
                            finish_q(o_pss[q], q)
                elif s < N_SUP - 1:
                    for c in range(KP):
                        s_mm(ah_sb, c, start=(c == 0), stop=(c == KP - 1))
                    sm_muls()
                    for q in range(N_SUB):
                        o_ps = pso.tile([P, D_OUT], F32, tag="ops")
                        for xt_sb, wt_sb in (
                            (xh_sb, wh_sb),
                            (xl_sb, wh_sb),
                            (xh_sb, wl_sb),
                        ):
                            for n in range(NB):
                                for c in range(KP):
                                    main_mm(o_ps, q, n, xt_sb, wt_sb, c,
                                            start=(xt_sb is xh_sb
                                                   and wt_sb is wh_sb
                                                   and c == 0))
                        for n in range(NB):
                            b_mm(o_ps, q, n)
                        finish_q(o_ps, q)
                else:
                    # Last supertile: n-major per q with per-half epilogues
                    # so output drains while the remaining matmuls run; the
                    # very last half descales in two quarters on DVE+Pool
                    # in parallel with stores on the idle sync queue, to
                    # minimize the after-last-matmul tail.
                    for c in range(KP):
                        s_mm(ah_sb, c, start=(c == 0), stop=(c == KP - 1))
                    sm_muls()
                    for q in range(N_SUB):
                        o_ps = pso.tile([P, D_OUT], F32, tag="ops")
                        if q < N_SUB - 1:
                            # steady order; only the last q runs n-major
                            for xt_sb, wt_sb in (
                                (xh_sb, wh_sb),
                                (xl_sb, wh_sb),
                                (xh_sb, wl_sb),
                            ):
                                for n in range(NB):
                                    for c in range(KP):
                                        main_mm(o_ps, q, n, xt_sb, wt_sb, c,
                                                start=(xt_sb is xh_sb
                                                       and wt_sb is wh_sb
                                                       and c == 0))
                        for n in range(NB):
                            if q == N_SUB - 1:
                                for xt_sb, wt_sb in (
                                    (xh_sb, wh_sb),
                                    (xl_sb, wh_sb),
                                    (xh_sb, wl_sb),
                                ):
                                    for c in range(KP):
                                        main_mm(o_ps, q, n, xt_sb, wt_sb, c,
                                                start=(xt_sb is xh_sb
                                                       and wt_sb is wh_sb
                                                       and c == 0))
                            b_mm(o_ps, q, n)
                            nhl = slice(n * 512, (n + 1) * 512)
                            if q == N_SUB - 1 and n == NB - 1:
                                for i in range(2):
                                    qsl4 = slice(n * 512 + i * 256,
                                                 n * 512 + (i + 1) * 256)
                                    if i == 0:
                                        nc.vector.tensor_scalar_mul(
                                            o_sb[:, q, qsl4], o_ps[:, qsl4],
                                            1.0 / SA,
                                        )
                                    else:
                                        nc.scalar.activation(
                                            o_sb[:, q, qsl4], o_ps[:, qsl4],
                                            mybir.ActivationFunctionType.Copy,
                                            scale=1.0 / SA,
                                        )
                                    nc.sync.dma_start(
                                        out=out_r[s][:, q, qsl4],
                                        in_=o_sb[:, q, qsl4],
                                    )
                            else:
                                if (q + n) % 2 == 0:
                                    nc.vector.tensor_scalar_mul(
                                        o_sb[:, q, nhl], o_ps[:, nhl], 1.0 / SA
                                    )
                                else:
                                    nc.scalar.activation(
                                        o_sb[:, q, nhl], o_ps[:, nhl],
                                        mybir.ActivationFunctionType.Copy,
                                        scale=1.0 / SA,
                                    )
                                nc.gpsimd.dma_start(
                                    out=out_r[s][:, q, nhl],
                                    in_=o_sb[:, q, nhl],
                                )

    nc.compile()
    return nc


_NC_CACHE = None


def _get_nc():
    global _NC_CACHE
    if _NC_CACHE is None:
        _NC_CACHE = build_bass()
    return _NC_CACHE


def make_in_maps(x, W, b, lora_A, lora_B, masks):
    x = np.ascontiguousarray(x, dtype=np.float32)
    W = np.ascontiguousarray(W, dtype=np.float32)
    b = np.ascontiguousarray(b, dtype=np.float32)
    lora_A = np.ascontiguousarray(lora_A, dtype=np.float32)
    lora_B = np.ascontiguousarray(lora_B, dtype=np.float32)
    masks = np.ascontiguousarray(masks, dtype=np.float32)

    x_flat = x.reshape(B * T, D_IN)
    x_hi8 = x_flat.astype(NPF8)                     # [BT, D_IN] fp8
    x_lo8 = (x_flat - x_hi8.astype(np.float32)).astype(NPF8)

    Wp = W.T.astype(np.float32) * np.float32(SA)    # [D_IN, D_OUT]
    wh8 = Wp.astype(NPF8)
    wl8 = (Wp - wh8.astype(np.float32)).astype(NPF8)
    wh8 = np.ascontiguousarray(wh8.reshape(KP, 2, P, D_OUT))
    wl8 = np.ascontiguousarray(wl8.reshape(KP, 2, P, D_OUT))

    A_flat = lora_A.reshape(NR, D_IN)
    Ap = A_flat.T.astype(np.float32) * np.float32(SA)  # [D_IN, NR]
    ah8 = np.ascontiguousarray(Ap.astype(NPF8).reshape(KP, 2, P, NR))

    B_flat = lora_B.transpose(1, 0, 2).reshape(D_OUT, NR)
    Bp = B_flat.T.astype(np.float32) * np.float32(SA)  # [NR, D_OUT]
    bth8 = Bp.astype(NPF8)
    btl8 = (Bp - bth8.astype(np.float32)).astype(NPF8)
    # btp rows 0..63: j0 = B_hi, j1 = B_lo. Rows 64/65: fp8 hi/lo of the
    # scaled bias, paired with sm rows memset to 1.0. Rows 66+: zero.
    btp8 = np.zeros((P, 2, D_OUT), dtype=NPF8)
    btp8[:NR, 0, :] = bth8
    btp8[:NR, 1, :] = btl8
    bp = b.astype(np.float32) * np.float32(SA)
    bh8 = bp.astype(NPF8)
    bl8 = (bp - bh8.astype(np.float32)).astype(NPF8)
    btp8[NR, 0, :] = bh8
    btp8[NR + 1, 0, :] = bl8
    btp8 = np.ascontiguousarray(btp8)

    # mask' = mask * 2 / SA so the product  s_psum * mask'  lands at
    # (2 * s * mask), exact powers of two -> fp8-exact.
    m_full = masks[..., 0].reshape(N_ADAPT, B * T)
    m_exp = np.repeat(m_full, R, axis=0) * np.float32(2.0 / SA)  # [NR, BT]
    m8 = m_exp.astype(NPF8)

    in_maps = []
    for c in range(N_CORES):
        sl = slice(c * TOK, (c + 1) * TOK)
        xh_c = np.ascontiguousarray(x_hi8[sl].T.reshape(KP, 2, P, TOK))
        xl_c = np.ascontiguousarray(x_lo8[sl].T.reshape(KP, 2, P, TOK))
        in_maps.append(
            {
                "xh": xh_c,
                "xl": xl_c,
                "wh": wh8,
                "wl": wl8,
                "ah": ah8,
                "btp": btp8,
                "msk": np.ascontiguousarray(m8[:, sl]),
            }
        )
    return in_maps


def kernel(x, W, b, lora_A, lora_B, masks):
    nc = _get_nc()
    in_maps = make_in_maps(x, W, b, lora_A, lora_B, masks)
    res = run_bass_kernel_spmd(nc, in_maps, core_ids=list(range(N_CORES)))
    out = np.concatenate([r["out"] for r in res.results], axis=0)
    out = out.reshape(B, T, D_OUT)
    return out


# revision 38
# speedup vs baseline: 1.5896x; 1.0008x over previous
"""Routed-LoRA linear layer (moe_routing) on 8 trn2 NeuronCores.

Math (per token t):
  out[t, :] = W @ x[t] + b + 2.0 * sum_n mask[n, t] * (B_n @ (A_n @ x[t]))

Strategy:
  - Data-parallel over B*T = 65536 tokens: 8192 tokens per core.
  - All heavy matmuls run in fp8e4m3 with perf_mode=DoubleRow (2 k-tiles
    packed per instruction, K=256 contraction per matmul, half cycle cost
    per output row vs full-rate dtypes).
  - Precision is recovered with an error-compensated split, all terms
    sharing a single 2^7 scale (on the W/A/B side) so they accumulate
    into one PSUM group:
      main : (x_hi + x_lo) @ W_hi + x_hi @ W_lo      (x_lo@W_lo dropped)
      s    : x_hi @ (A_hi + A_lo)                     (rank-64 fused A)
      delta: sm @ (B_hi + B_lo)  -- B hi/lo ride the two DoubleRow j-slots
    where *_hi = fp8(v), *_lo = fp8(v - v_hi). sm = (s * mask) quantized
    to fp8 on the fly by the Activation-engine mask-multiply.
  - The bias rides two extra contraction rows of the delta matmul
    (sm rows 64/65 memset to 1.0, btp rows 64/65 hold fp8 hi/lo of
    2^7 * b), so the epilogue is a single per-q DVE descale copy.
"""

import numpy as np
import ml_dtypes

import concourse.bass as bass
from concourse import bacc
import concourse.mybir as mybir
import concourse.tile as tile
from concourse.bass_utils import run_bass_kernel_spmd

N_CORES = 8
B, T = 8, 8192
D_IN = 1024
D_OUT = 1024
N_ADAPT, R = 4, 16
NR = N_ADAPT * R  # 64
SCALING = 32.0 / 16.0

TOK = B * T // N_CORES  # 8192 tokens per core
SUP = 512               # tokens per supertile
N_SUP = TOK // SUP      # 16
SUB = 128               # tokens per matmul M-tile
N_SUB = SUP // SUB      # 4
P = 128
KP = D_IN // 256        # 4 DoubleRow contraction chunks (256 each)
NB = D_OUT // 512       # 2 PSUM-bank column halves

SA = 2.0 ** 7           # weight-side scale so W/A/B fp8 values are ~N(0, 2.56)

F32 = mybir.dt.float32
BF16 = mybir.dt.bfloat16
F8 = mybir.dt.float8e4
NPF8 = ml_dtypes.float8_e4m3
DR = mybir.MatmulPerfMode.DoubleRow


def build_bass():
    nc = bacc.Bacc(
        "TRN2", target_bir_lowering=False, debug=False, num_devices=N_CORES
    )

    xh_d = nc.dram_tensor("xh", [KP, 2, P, TOK], F8, kind="ExternalInput")
    xl_d = nc.dram_tensor("xl", [KP, 2, P, TOK], F8, kind="ExternalInput")
    wh_d = nc.dram_tensor("wh", [KP, 2, P, D_OUT], F8, kind="ExternalInput")
    wl_d = nc.dram_tensor("wl", [KP, 2, P, D_OUT], F8, kind="ExternalInput")
    ah_d = nc.dram_tensor("ah", [KP, 2, P, NR], F8, kind="ExternalInput")
    btp_d = nc.dram_tensor("btp", [P, 2, D_OUT], F8, kind="ExternalInput")
    msk_d = nc.dram_tensor("msk", [NR, TOK], F8, kind="ExternalInput")
    out_d = nc.dram_tensor("out", [TOK, D_OUT], F32, kind="ExternalOutput")

    xh_r = xh_d.ap().rearrange("c j p t -> p c j t")
    xl_r = xl_d.ap().rearrange("c j p t -> p c j t")
    wh_r = wh_d.ap().rearrange("c j p n -> p c j n")
    wl_r = wl_d.ap().rearrange("c j p n -> p c j n")
    ah_r = ah_d.ap().rearrange("c j p r -> p c j r")
    out_r = out_d.ap().rearrange("(s q p) n -> s p q n", q=N_SUB, p=P)

    with tile.TileContext(nc) as tc:
        with (
            tc.tile_pool(name="const", bufs=1) as const,
            tc.tile_pool(name="xp", bufs=3) as xp,
            tc.tile_pool(name="op", bufs=2) as op,
            tc.tile_pool(name="pss", bufs=2, space="PSUM") as pss,
            tc.tile_pool(name="pso", bufs=3, space="PSUM") as pso,
        ):
            wh_sb = const.tile([P, KP, 2, D_OUT], F8)
            wl_sb = const.tile([P, KP, 2, D_OUT], F8)
            ah_sb = const.tile([P, KP, 2, NR], F8)
            btp_sb = const.tile([P, 2, D_OUT], F8)
            m_sb = const.tile([NR, TOK], F8)
            sm_sb = const.tile([P, 2, SUP], F8)

            # Rows 64..127 of sm pair with btp rows 64/65 (the fp8 hi/lo of
            # 2^7 * bias; rows 66+ are zero), adding the bias inside the
            # delta matmul for free.
            nc.vector.memset(sm_sb[NR:P, :, :], 1.0)

            # DMA ordering: all transfers serialize on one shared engine
            # pool in readiness order, and supertile 0 consumes chunks in
            # exactly this arrival order: xh0, wh0, ah, wl0, al, wh1, xl0,
            # wl1, msk0, btp, wh2, wl2, wh3, wl3. Supertile 0's own x/A/msk
            # ride the otherwise-idle sync queue; later supertiles' x/msk
            # are prefetched from the scalar queue inside the previous
            # body, where the stores' sem-waits throttle their readiness
            # so they can't cut ahead of the remaining W chunks.
            for c in range(2):
                nc.scalar.dma_start(out=wh_sb[:, c, :, :], in_=wh_r[:, c, :, :])
                nc.scalar.dma_start(out=wl_sb[:, c, :, :], in_=wl_r[:, c, :, :])
            nc.scalar.dma_start(out=btp_sb[:], in_=btp_d.ap())
            for c in range(2, KP):
                nc.scalar.dma_start(out=wh_sb[:, c, :, :], in_=wh_r[:, c, :, :])
                nc.scalar.dma_start(out=wl_sb[:, c, :, :], in_=wl_r[:, c, :, :])

            x_tiles = {}

            def prefetch(s, queue):
                t0 = s * SUP
                tsl = slice(t0, t0 + SUP)
                xh_t = xp.tile([P, KP, 2, SUP], F8, tag="xh", name=f"xh{s}")
                xl_t = xp.tile([P, KP, 2, SUP], F8, tag="xl", name=f"xl{s}")
                x_tiles[s] = (xh_t, xl_t)
                if s == 0:
                    # split the cold-start x loads so the first matmul's
                    # chunk (k-pair 0) lands one pool-slot earlier
                    queue.dma_start(out=xh_t[:, :2], in_=xh_r[:, :2, :, tsl])
                    queue.dma_start(out=xh_t[:, 2:], in_=xh_r[:, 2:, :, tsl])
                    queue.dma_start(out=ah_sb[:], in_=ah_r)
                    queue.dma_start(out=xl_t[:, :2], in_=xl_r[:, :2, :, tsl])
                    queue.dma_start(out=xl_t[:, 2:], in_=xl_r[:, 2:, :, tsl])
                else:
                    queue.dma_start(out=xh_t[:], in_=xh_r[:, :, :, tsl])
                    queue.dma_start(out=xl_t[:], in_=xl_r[:, :, :, tsl])
                queue.dma_start(out=m_sb[:, tsl], in_=msk_d.ap()[:, tsl])

            prefetch(0, nc.sync)

            for s in range(N_SUP):
                t0 = s * SUP
                tsl = slice(t0, t0 + SUP)
                xh_sb, xl_sb = x_tiles.pop(s)
                if s + 1 < N_SUP:
                    prefetch(s + 1, nc.scalar)

                s_ps = pss.tile([NR, SUP], F32, tag="sps")
                o_sb = op.tile([P, N_SUB, D_OUT], F32, tag="o")

                def s_mm(a_sb, c, start, stop):
                    nc.tensor.matmul(
                        s_ps[:],
                        a_sb[:, c, :, :],
                        xh_sb[:, c, :, :],
                        start=start,
                        stop=stop,
                        perf_mode=DR,
                    )

                def main_mm(o_ps, q, n, xt_sb, wt_sb, c, start):
                    nsl = slice(n * 512, (n + 1) * 512)
                    qsl = slice(q * SUB, q * SUB + SUB)
                    nc.tensor.matmul(
                        o_ps[:, nsl],
                        xt_sb[:, c, :, qsl],
                        wt_sb[:, c, :, nsl],
                        start=start,
                        stop=False,
                        perf_mode=DR,
                        skip_group_check=True,
                    )

                def b_mm(o_ps, q, n):
                    nsl = slice(n * 512, (n + 1) * 512)
                    qsl = slice(q * SUB, q * SUB + SUB)
                    nc.tensor.matmul(
                        o_ps[:, nsl],
                        sm_sb[:, :, qsl],
                        btp_sb[:, :, nsl],
                        start=False,
                        stop=True,
                        perf_mode=DR,
                        skip_group_check=True,
                    )

                def sm_muls():
                    # sm = s * mask' quantized to fp8, written into both
                    # DoubleRow j-slots (they pair with bt_hi / bt_lo).
                    nc.vector.tensor_mul(sm_sb[:NR, 0, :], s_ps[:], m_sb[:, tsl])
                    nc.vector.tensor_mul(sm_sb[:NR, 1, :], s_ps[:], m_sb[:, tsl])

                def finish_q(o_ps, q):
                    # out = psum * 2^-7 (bias already accumulated); per-q
                    # stores shorten the kernel tail. Descales alternate
                    # DVE / gpsimd so the DVE queue stays shallow for the
                    # sm multiplies. The very last supertile drains in
                    # fine-grained slices so the final descale->store chain
                    # after the last matmul is as short as possible.
                    for i in range(1):
                        nsl = slice(0, D_OUT)
                        nc.vector.tensor_scalar_mul(
                            o_sb[:, q, nsl], o_ps[:, nsl], 1.0 / SA
                        )
                        nc.gpsimd.dma_start(
                            out=out_r[s][:, q, nsl], in_=o_sb[:, q, nsl]
                        )

                if s == 0:
                    # Supertile 0 is fed by a cold DMA pipe: consume chunks
                    # strictly in arrival order (xh0, wh0, ah, wl0, xl0,
                    # wh1, wl1, msk0, btp, wh2, wl2, wh3, wl3), c-outer
                    # over a q-triple so each W chunk unlocks ~640ns of
                    # matmuls the moment it lands (chunks arrive every
                    # ~730ns). q3 runs as round B with all data resident.
                    for qgrp in ((0, 1), (2, 3)):
                        o_pss = {}
                        for q in qgrp:
                            o_pss[q] = pso.tile(
                                [P, D_OUT], F32, tag="ops",
                                name=f"ops_s0_q{q}",
                            )

                        def grp_term(xt_sb, wt_sb, c, start=False):
                            for q in qgrp:
                                for n in range(NB):
                                    main_mm(o_pss[q], q, n, xt_sb, wt_sb, c,
                                            start=start)

                        if qgrp[0] == 0:
                            grp_term(xh_sb, wh_sb, 0, start=True)    # wh0
                            for c in range(KP):                      # ah
                                s_mm(ah_sb, c, start=(c == 0),
                                     stop=(c == KP - 1))
                            grp_term(xh_sb, wl_sb, 0)                # wl0
                            grp_term(xl_sb, wh_sb, 0)                # xl0
                            grp_term(xh_sb, wh_sb, 1)                # wh1
                            grp_term(xl_sb, wh_sb, 1)
                            grp_term(xh_sb, wl_sb, 1)                # wl1
                            sm_muls()                                # msk0
                            grp_term(xh_sb, wh_sb, 2)                # wh2
                            grp_term(xl_sb, wh_sb, 2)
                            grp_term(xh_sb, wl_sb, 2)                # wl2
                            grp_term(xh_sb, wh_sb, 3)                # wh3
                            grp_term(xl_sb, wh_sb, 3)
                            grp_term(xh_sb, wl_sb, 3)                # wl3
                        else:
                            grp_term(xh_sb, wh_sb, 0, start=True)
                            for c in range(1, KP):
                                grp_term(xh_sb, wh_sb, c)
                            for c in range(KP):
                                grp_term(xl_sb, wh_sb, c)
                            for c in range(KP):
                                grp_term(xh_sb, wl_sb, c)
                        for q in qgrp:
                            for n in range(NB):
                                b_mm(o_pss[q], q, n)                 # btp
                        for q in qgrp:
                            finish_q(o_pss[q], q)
                elif s < N_SUP - 1:
                    for c in range(KP):
                        s_mm(ah_sb, c, start=(c == 0), stop=(c == KP - 1))
                    sm_muls()
                    for q in range(N_SUB):
                        o_ps = pso.tile([P, D_OUT], F32, tag="ops")
                        for xt_sb, wt_sb in (
                            (xh_sb, wh_sb),
                            (xl_sb, wh_sb),
                            (xh_sb, wl_sb),
                        ):
                            for n in range(NB):
                                for c in range(KP):
                                    main_mm(o_ps, q, n, xt_sb, wt_sb, c,
                                            start=(xt_sb is xh_sb
                                                   and wt_sb is wh_sb
                                                   and c == 0))
                        for n in range(NB):
                            b_mm(o_ps, q, n)
                        finish_q(o_ps, q)
                else:
                    # Last supertile: n-major per q with per-half epilogues
                    # so output drains while the remaining matmuls run; the
                    # very last half descales in two quarters on DVE+Pool
                    # in parallel with stores on the idle sync queue, to
                    # minimize the after-last-matmul tail.
                    for c in range(KP):
                        s_mm(ah_sb, c, start=(c == 0), stop=(c == KP - 1))
                    sm_muls()
                    for q in range(N_SUB):
                        o_ps = pso.tile([P, D_OUT], F32, tag="ops")
                        if q < N_SUB - 1:
                            # steady order; only the last q runs n-major
                            for xt_sb, wt_sb in (
                                (xh_sb, wh_sb),
                                (xl_sb, wh_sb),
                                (xh_sb, wl_sb),
                            ):
                                for n in range(NB):
                                    for c in range(KP):
                                        main_mm(o_ps, q, n, xt_sb, wt_sb, c,
                                                start=(xt_sb is xh_sb
                                                       and wt_sb is wh_sb
                                                       and c == 0))
                        for n in range(NB):
                            if q == N_SUB - 1:
                                for xt_sb, wt_sb in (
                                    (xh_sb, wh_sb),
                                    (xl_sb, wh_sb),
                                    (xh_sb, wl_sb),
                                ):
                                    for c in range(KP):
                                        main_mm(o_ps, q, n, xt_sb, wt_sb, c,
                                                start=(xt_sb is xh_sb
                                                       and wt_sb is wh_sb
                                                       and c == 0))
                            b_mm(o_ps, q, n)
                            nhl = slice(n * 512, (n + 1) * 512)
                            if q == N_SUB - 1 and n == NB - 1:
                                for i in range(2):
                                    qsl4 = slice(n * 512 + i * 256,
                                                 n * 512 + (i + 1) * 256)
                                    if i == 0:
                                        nc.vector.tensor_scalar_mul(
                                            o_sb[:, q, qsl4], o_ps[:, qsl4],
                                            1.0 / SA,
                                        )
                                    else:
                                        nc.scalar.activation(
                                            o_sb[:, q, qsl4], o_ps[:, qsl4],
                                            mybir.ActivationFunctionType.Copy,
                                            scale=1.0 / SA,
                                        )
                                    nc.sync.dma_start(
                                        out=out_r[s][:, q, qsl4],
                                        in_=o_sb[:, q, qsl4],
                                    )
                            else:
                                nc.vector.tensor_scalar_mul(
                                    o_sb[:, q, nhl], o_ps[:, nhl], 1.0 / SA
                                )
                                nc.gpsimd.dma_start(
                                    out=out_r[s][:, q, nhl],
                                    in_=o_sb[:, q, nhl],
                                )

    nc.compile()
    return nc


_NC_CACHE = None


def _get_nc():
    global _NC_CACHE
    if _NC_CACHE is None:
        _NC_CACHE = build_bass()
    return _NC_CACHE


def make_in_maps(x, W, b, lora_A, lora_B, masks):
    x = np.ascontiguousarray(x, dtype=np.float32)
    W = np.ascontiguousarray(W, dtype=np.float32)
    b = np.ascontiguousarray(b, dtype=np.float32)
    lora_A = np.ascontiguousarray(lora_A, dtype=np.float32)
    lora_B = np.ascontiguousarray(lora_B, dtype=np.float32)
    masks = np.ascontiguousarray(masks, dtype=np.float32)

    x_flat = x.reshape(B * T, D_IN)
    x_hi8 = x_flat.astype(NPF8)                     # [BT, D_IN] fp8
    x_lo8 = (x_flat - x_hi8.astype(np.float32)).astype(NPF8)

    Wp = W.T.astype(np.float32) * np.float32(SA)    # [D_IN, D_OUT]
    wh8 = Wp.astype(NPF8)
    wl8 = (Wp - wh8.astype(np.float32)).astype(NPF8)
    wh8 = np.ascontiguousarray(wh8.reshape(KP, 2, P, D_OUT))
    wl8 = np.ascontiguousarray(wl8.reshape(KP, 2, P, D_OUT))

    A_flat = lora_A.reshape(NR, D_IN)
    Ap = A_flat.T.astype(np.float32) * np.float32(SA)  # [D_IN, NR]
    ah8 = np.ascontiguousarray(Ap.astype(NPF8).reshape(KP, 2, P, NR))

    B_flat = lora_B.transpose(1, 0, 2).reshape(D_OUT, NR)
    Bp = B_flat.T.astype(np.float32) * np.float32(SA)  # [NR, D_OUT]
    bth8 = Bp.astype(NPF8)
    btl8 = (Bp - bth8.astype(np.float32)).astype(NPF8)
    # btp rows 0..63: j0 = B_hi, j1 = B_lo. Rows 64/65: fp8 hi/lo of the
    # scaled bias, paired with sm rows memset to 1.0. Rows 66+: zero.
    btp8 = np.zeros((P, 2, D_OUT), dtype=NPF8)
    btp8[:NR, 0, :] = bth8
    btp8[:NR, 1, :] = btl8
    bp = b.astype(np.float32) * np.float32(SA)
    bh8 = bp.astype(NPF8)
    bl8 = (bp - bh8.astype(np.float32)).astype(NPF8)
    btp8[NR, 0, :] = bh8
    btp8[NR + 1, 0, :] = bl8
    btp8 = np.ascontiguousarray(btp8)

    # mask' = mask * 2 / SA so the product  s_psum * mask'  lands at
    # (2 * s * mask), exact powers of two -> fp8-exact.
    m_full = masks[..., 0].reshape(N_ADAPT, B * T)
    m_exp = np.repeat(m_full, R, axis=0) * np.float32(2.0 / SA)  # [NR, BT]
    m8 = m_exp.astype(NPF8)

    in_maps = []
    for c in range(N_CORES):
        sl = slice(c * TOK, (c + 1) * TOK)
        xh_c = np.ascontiguousarray(x_hi8[sl].T.reshape(KP, 2, P, TOK))
        xl_c = np.ascontiguousarray(x_lo8[sl].T.reshape(KP, 2, P, TOK))
        in_maps.append(
            {
                "xh": xh_c,
                "xl": xl_c,
                "wh": wh8,
                "wl": wl8,
                "ah": ah8,
                "btp": btp8,
                "msk": np.ascontiguousarray(m8[:, sl]),
            }
        )
    return in_maps


def kernel(x, W, b, lora_A, lora_B, masks):
    nc = _get_nc()
    in_maps = make_in_maps(x, W, b, lora_A, lora_B, masks)
    res = run_bass_kernel_spmd(nc, in_maps, core_ids=list(range(N_CORES)))
    out = np.concatenate([r["out"] for r in res.results], axis=0)
    out = out.reshape(B, T, D_OUT)
    return out
